# revision 12
# baseline (speedup 1.0000x reference)
"""Multi-head causal attention (B=4, L=2048, E=1024, H=16) on 8 trn2 NeuronCores.

Sharding: (batch, head-group) grid — core c handles batch b=c//2 and heads
g=c%2 (8 heads each).  Each core computes its heads' QKV projection, causal
attention, and a partial output projection; the host sums the two partials
per batch.

v5: on-chip softmax-denominator broadcast, DMA-lean startup/tail:
  - all matmul operands bf16 (PSUM stays fp32); heads 2m/2m+1 paired in
    disjoint PE row halves for scores; one ACTIVATE exps both heads per
    (window, lk-tile); v carries a ones column so PV also emits the
    denominator row (pv row 64).
  - normalization is now fully on-chip: denominator row -> K=1 matmul
    outer-product broadcast (ones[1,64].T @ denom[1,512]) into a heater
    PSUM tile -> DVE reciprocal [64,512] -> tensor_mul.  Replaces v4's
    two DRAM round trips + 4 DMA issues per window-head (~128 DMA issues,
    ~100us of sync/gpsimd engine time) and shortens the dependency chain
    so the tail collapses.  The chain is emitted as 4 deferred closures
    popped one per later-t so no engine FIFO ever head-of-line blocks.
  - startup: PE warms on a memset tile from ~0.5us (no DMA dependency);
    only xc0+xc1+wvT+wqk are fetched up front (5MB/core; all 8 cores share
    HBM so startup is HBM-bound) on the two HWDGE rings; xc2/xc3 are
    fetched mid-A-phase via sync-ring heater closures; wqkT is split into
    q/k halves so qk_unit(0/4) can start after the q half lands.
  - y partials are written bf16 (halves the 8MB/core output traffic);
    host sums in fp32.
"""

import numpy as np

L = 2048
E = 1024
NH = 8        # heads per core
D = 64
JQ = 512      # feature rows per core (NH*D)
ET = E // 128  # 8 e-tiles
LT = L // 128  # 16 l-tiles

_CACHE = {}
DEBUG_DUMP = False
DEBUG_FULL_SCORES = False  # compute full score rectangles (race-detector aid)


def build_nc():
    import concourse.mybir as mybir
    import concourse.tile as tile
    from concourse import bacc
    from contextlib import ExitStack

    f32 = mybir.dt.float32
    bf16 = mybir.dt.bfloat16
    Exp = mybir.ActivationFunctionType.Exp

    nc = bacc.Bacc("TRN2", target_bir_lowering=False, debug=False)

    # all inputs host-pre-shuffled to SBUF layout (partition dim first)
    xT_d = [nc.declare_dram_parameter(f"xT{c}", [128, ET, 512], bf16, isOutput=False)
            for c in range(4)]
    wqkT_d = nc.declare_dram_parameter("wqkT", [128, 2, ET, JQ], bf16, isOutput=False)
    wvT_d = nc.declare_dram_parameter("wvT", [128, ET, JQ], bf16, isOutput=False)
    woT_d = nc.declare_dram_parameter("woT", [128, 4, E], bf16, isOutput=False)
    diag_d = nc.declare_dram_parameter("diag", [128, 128], bf16, isOutput=False)
    y_d = nc.declare_dram_parameter("y", [L, E], bf16, isOutput=True)

    with ExitStack() as ctx:
        tc = ctx.enter_context(tile.TileContext(nc))

        consts = ctx.enter_context(tc.tile_pool(name="consts", bufs=1))
        warm_sb = consts.tile([128, 128], bf16, name="warm_sb")
        nc.vector.memset(warm_sb, 0.0)
        ones_sb = consts.tile([65, 64], bf16, name="ones_sb")
        nc.vector.memset(ones_sb, 1.0)
        diag_sb = consts.tile([128, 128], bf16, name="diag_sb")
        nc.sync.dma_start(out=diag_sb, in_=diag_d.ap())

        # startup is HBM-bound (all 8 cores fetch at once): land tensors in
        # the order compute consumes them.  sync ring: xc0, xc1; act ring:
        # wvT, then q weights, then k weights.  xc2/xc3 are deferred to
        # A-phase heater closures (sync ring).
        xT_p = ctx.enter_context(tc.tile_pool(name="xT", bufs=1))
        xcs = [xT_p.tile([128, ET, 512], bf16, tag=f"xc{c}", name=f"xc{c}")
               for c in range(4)]
        nc.sync.dma_start(out=xcs[0], in_=xT_d[0].ap())
        nc.sync.dma_start(out=xcs[1], in_=xT_d[1].ap())
        wv_p = ctx.enter_context(tc.tile_pool(name="wv", bufs=1))
        wvT_sb = wv_p.tile([128, ET, JQ], bf16)         # 8KB/part
        nc.scalar.dma_start(out=wvT_sb, in_=wvT_d.ap())
        wqk_p = ctx.enter_context(tc.tile_pool(name="wqk", bufs=1))
        wqkT_sb = wqk_p.tile([128, 2, ET, JQ], bf16)    # 16KB/part
        nc.scalar.dma_start(out=wqkT_sb[:, 0], in_=wqkT_d.ap()[:, 0])
        nc.scalar.dma_start(out=wqkT_sb[:, 1], in_=wqkT_d.ap()[:, 1])

        vaug_p = ctx.enter_context(tc.tile_pool(name="vaug", bufs=1))
        v_aug = vaug_p.tile([128, LT, NH, 65], bf16)    # 16.6KB/part
        nc.vector.memset(v_aug[:, :, :, 64:65], 1.0)

        qk_p = ctx.enter_context(tc.tile_pool(name="qk", bufs=1))
        qT_sb = qk_p.tile([128, 4, L], bf16)            # 16KB/part
        kT_sb = qk_p.tile([128, 4, L], bf16)            # 16KB/part
        ao_p = ctx.enter_context(tc.tile_pool(name="ao", bufs=1))
        aoT_sb = ao_p.tile([128, 4, L], bf16)           # 16KB/part

        sc_pp = ctx.enter_context(tc.tile_pool(name="scpp", bufs=2, space="PSUM"))
        pv_pp = ctx.enter_context(tc.tile_pool(name="pvpp", bufs=2, space="PSUM"))
        hp_pp = ctx.enter_context(tc.tile_pool(name="hppp", bufs=2, space="PSUM"))
        pt_p = ctx.enter_context(tc.tile_pool(name="pt", bufs=12))
        rc_p = ctx.enter_context(tc.tile_pool(name="rc", bufs=4))
        aou_p = ctx.enter_context(tc.tile_pool(name="aou", bufs=6))
        y_p = ctx.enter_context(tc.tile_pool(name="y", bufs=4))

        dq = []  # deferred closures, popped one per attention t

        # ---- projection / outproj units (heater pool; 1 PSUM bank each) ----
        def v_unit(c, i):
            ps = hp_pp.tile([128, 512], f32, tag="hp", name="vps")
            for et in range(ET):
                nc.tensor.matmul(
                    ps,
                    lhsT=xcs[c][:, et, i * 128:(i + 1) * 128],
                    rhs=wvT_sb[:, et, :],
                    start=(et == 0), stop=(et == ET - 1),
                )
            nc.vector.tensor_copy(
                out=v_aug[:, c * 4 + i, :, 0:64],
                in_=ps.rearrange("p (h d) -> p h d", h=NH),
            )

        def qk_unit(jt, c):
            # jt 0..3 = q j-tiles, 4..7 = k j-tiles
            ps = hp_pp.tile([128, 512], f32, tag="hp", name="qkps")
            qk = 0 if jt < 4 else 1
            dst = qT_sb if jt < 4 else kT_sb
            for et in range(ET):
                nc.tensor.matmul(
                    ps,
                    lhsT=wqkT_sb[:, qk, et, (jt % 4) * 128:(jt % 4 + 1) * 128],
                    rhs=xcs[c][:, et, :],
                    start=(et == 0), stop=(et == ET - 1),
                )
            nc.vector.tensor_copy(out=dst[:, jt % 4, c * 512:(c + 1) * 512], in_=ps)

        def op_unit(lt, ec, ring=None):
            ps = hp_pp.tile([128, 512], f32, tag="hp", name="opps")
            for jt in range(4):
                nc.tensor.matmul(
                    ps,
                    lhsT=aoT_sb[:, jt, lt * 128:(lt + 1) * 128],
                    rhs=woT_sb[:, jt, ec * 512:(ec + 1) * 512],
                    start=(jt == 0), stop=(jt == 3),
                )
            yt = y_p.tile([128, 512], bf16, tag="y")
            nc.vector.tensor_copy(out=yt, in_=ps)
            (ring or nc.gpsimd).dma_start(
                out=y_d.ap()[lt * 128:(lt + 1) * 128, ec * 512:(ec + 1) * 512],
                in_=yt,
            )

        # ---- attention ------------------------------------------------
        def pair_unit(m, phase, hw0=(), hw1=(), last=False):
            """Heads (2m, 2m+1); phase 0 = lq windows 0,1; phase 1 = windows 2,3.

            Head A (partitions 0:64) scores land in sc[:, 0:512], head B
            (64:128) in sc[:, 512:1024]; one ACTIVATE exps both.  heaters
            (hw0/hw1 per window) are drained one per t into their own PSUM
            pool, filling PE slack under the exp stream.
            """
            for w, heaters in ((2 * phase, hw0), (2 * phase + 1, hw1)):
                lq0 = w * 512
                nt = 4 * w + 4
                pvA = pv_pp.tile([65, 512], f32, tag="pv", name="pvA")
                pvB = pv_pp.tile([65, 512], f32, tag="pv", name="pvB")
                hq = list(heaters)
                # PV trails scores by TWO tiles so the first PV of this
                # window issues after the previous window's pv readers have
                # released the slots (else it blocks the PE FIFO)
                pend = []

                def emit_pv(p, stop):
                    pe, poff, tt = p
                    for pv, base, h in ((pvA, 0, 2 * m), (pvB, 512, 2 * m + 1)):
                        nc.tensor.matmul(
                            pv[:, poff:512],
                            lhsT=v_aug[:, tt, h, :],
                            rhs=pe[:, base + poff:base + 512],
                            start=(tt == 0), stop=stop,
                            skip_group_check=True,
                        )

                for t in range(nt):
                    off = max(0, t * 128 - lq0)
                    moff = 0 if DEBUG_FULL_SCORES else off
                    sc = sc_pp.tile([128, 1024], f32, tag="sc", name="sc")
                    for po, base in ((0, 0), (64, 512)):
                        nc.tensor.matmul(
                            sc[:, base + moff:base + 512],
                            lhsT=kT_sb[po:po + 64, m, t * 128:(t + 1) * 128],
                            rhs=qT_sb[po:po + 64, m, lq0 + moff:lq0 + 512],
                            start=True, stop=True,
                        )
                    pe = pt_p.tile([128, 1024], bf16, tag="pe", name="pe")
                    nc.scalar.activation(out=pe[:, moff:1024], in_=sc[:, moff:1024],
                                         func=Exp, scale=0.125)
                    if t >= 4 * w:  # diagonal block: zero lk > lq
                        for base in (0, 512):
                            nc.vector.tensor_mul(
                                out=pe[:, base + off:base + off + 128],
                                in0=pe[:, base + off:base + off + 128],
                                in1=diag_sb,
                            )
                    if dq:
                        dq.pop(0)()
                    if hq:
                        hq.pop(0)()
                    if len(pend) == 2:
                        emit_pv(pend.pop(0), stop=False)
                    pend.append((pe, off, t))
                while pend:
                    emit_pv(pend.pop(0), stop=(len(pend) == 0))
                # normalize, fully on-chip: copy pv -> sbuf (bf16), broadcast
                # the denominator row to 64 partitions with a K=1 matmul,
                # reciprocal, multiply.  Emitted as deferred closures popped
                # in LATER windows' t-loops so no FIFO head-of-line blocks on
                # a cross-engine wait.
                aoUs, dens, rcbs = [], [], []
                for pv, nm in ((pvA, "A"), (pvB, "B")):
                    aoU = aou_p.tile([65, 512], bf16, tag="aou", name="aoU" + nm)
                    nc.vector.tensor_copy(out=aoU, in_=pv)
                    aoUs.append(aoU)

                # den tiles come from the PV pool: its slot-reuse WAR is
                # already decoupled from the PE FIFO by the pend-2 delay, so
                # the next window's first PV never stalls on the reciprocal.
                def bcast(aoUs=aoUs, dens=dens):
                    for aoU in aoUs:
                        ps = pv_pp.tile([65, 512], f32, tag="pv", name="denps")
                        nc.tensor.matmul(
                            ps[0:64, :], lhsT=ones_sb[64:65, :],
                            rhs=aoU[64:65, :], start=True, stop=True,
                        )
                        dens.append(ps)

                def recip(dens=dens, rcbs=rcbs):
                    for ps in dens:
                        rcb = rc_p.tile([64, 512], f32, tag="rcb", name="rcb")
                        nc.vector.reciprocal(out=rcb, in_=ps[0:64, :])
                        rcbs.append(rcb)

                def mk_mul(i, po):
                    def mul(aoUs=aoUs, rcbs=rcbs, po=po, i=i, m=m, lq0=lq0):
                        nc.vector.tensor_mul(
                            out=aoT_sb[po:po + 64, m, lq0:lq0 + 512],
                            in0=aoUs[i][0:64, :], in1=rcbs[i],
                        )
                    return mul

                if last and w == 3:
                    bcast(); recip(); mk_mul(0, 0)(); mk_mul(1, 64)()
                else:
                    dq.append(bcast)
                    dq.append(recip)
                    dq.append(mk_mul(0, 0))
                    dq.append(mk_mul(1, 64))

        # ---- schedule -------------------------------------------------
        # warm the PE pstate on the memset tile from ~0.5us (no DMA dep);
        # the trailing memsets zero the sc slots so diagonal-strip exps
        # never see raw PSUM
        warmA = sc_pp.tile([128, 1024], f32, tag="sc", name="warmA")
        for _ in range(40):
            nc.tensor.matmul(
                warmA[:, 0:128], lhsT=warm_sb, rhs=warm_sb,
                start=True, stop=True, skip_group_check=True,
            )
        nc.vector.memset(warmA, 0.0)
        warmB = sc_pp.tile([128, 1024], f32, tag="sc", name="warmB")
        nc.vector.memset(warmB, 0.0)

        # P0: only pair-0-window-0's prerequisites run serially; everything
        # else overlaps attention as heaters.  v first (needs xc0+wvT, the
        # earliest arrivals), then q/k j-tile 0 as the weight halves land.
        for i in range(4):
            v_unit(0, i)
        qk_unit(0, 0)
        qk_unit(4, 0)

        QK = lambda jt, c: (lambda: qk_unit(jt, c))
        VU = lambda c, i: (lambda: v_unit(c, i))
        OP = lambda lt, ec: (lambda: op_unit(lt, ec))
        DX = lambda c: (lambda: nc.sync.dma_start(out=xcs[c], in_=xT_d[c].ap()))

        # A-phase: windows 0,1; heaters finish the q/k projection and kick
        # off the deferred xc2/xc3 fetches on the idle sync ring.  QK(0,1)/
        # QK(4,1) MUST run in window 0: pair-0-window-1's own scores read
        # their qT/kT columns from t=0 (xc1 lands mid-P0, well before w0-t0).
        pair_unit(0, 0, [QK(0, 1), QK(4, 1), QK(1, 0), QK(5, 0)],
                  [VU(1, 0), VU(1, 1), VU(1, 2), VU(1, 3),
                   QK(1, 1), QK(5, 1), DX(2), DX(3)])
        pair_unit(1, 0, [QK(6, 0), QK(2, 0)], [QK(6, 1), QK(2, 1)])
        pair_unit(2, 0, [QK(7, 0), QK(3, 0)], [QK(7, 1), QK(3, 1)])
        pair_unit(3, 0, [QK(0, 2), QK(0, 3), QK(4, 2)],
                  [QK(4, 3), QK(1, 2), QK(1, 3), QK(5, 2), QK(5, 3)])

        # B-phase: windows 2,3; heaters: v for lk>=1024, remaining q/k,
        # then the output projection as soon as its aoT rows are final
        woT_sb = wqk_p.tile([128, 4, E], bf16, tag="wqkT_sb", name="woT_sb")

        def load_wo():
            nc.gpsimd.dma_start(out=woT_sb, in_=woT_d.ap())

        pair_unit(0, 1, [VU(2, 0), VU(2, 1), VU(2, 2), VU(2, 3)],
                  [VU(3, 0), VU(3, 1), VU(3, 2), VU(3, 3)])
        pair_unit(1, 1, [QK(2, 2), QK(2, 3), QK(6, 2)], [QK(6, 3)])
        pair_unit(2, 1, [QK(3, 2), QK(3, 3), QK(7, 2)],
                  [QK(7, 3), load_wo, OP(0, 0), OP(0, 1), OP(1, 0), OP(1, 1)])
        pair_unit(3, 1, [OP(2, 0), OP(2, 1), OP(3, 0), OP(3, 1),
                         OP(4, 0), OP(4, 1), OP(5, 0), OP(5, 1)],
                  [OP(6, 0), OP(6, 1), OP(7, 0), OP(7, 1),
                   OP(8, 0), OP(8, 1), OP(9, 0), OP(9, 1),
                   OP(10, 0), OP(10, 1), OP(11, 0), OP(11, 1)],
                  last=True)

        # drain deferred closures, then the tail of the output projection
        while dq:
            dq.pop(0)()
        for lt in range(12, LT):
            for ec in range(2):
                op_unit(lt, ec, ring=(nc.sync if ec == 0 else nc.scalar))

        if DEBUG_DUMP:
            dbg_ao = nc.declare_dram_parameter("dbg_ao", [128, 4, L], bf16, isOutput=True)
            dbg_q = nc.declare_dram_parameter("dbg_q", [128, 4, L], bf16, isOutput=True)
            dbg_k = nc.declare_dram_parameter("dbg_k", [128, 4, L], bf16, isOutput=True)
            dbg_v = nc.declare_dram_parameter("dbg_v", [128, LT, NH, 65], bf16, isOutput=True)
            nc.sync.dma_start(out=dbg_ao.ap(), in_=aoT_sb)
            nc.sync.dma_start(out=dbg_q.ap(), in_=qT_sb)
            nc.sync.dma_start(out=dbg_k.ap(), in_=kT_sb)
            nc.sync.dma_start(out=dbg_v.ap(), in_=v_aug)

    nc.compile()
    return nc


def _shuf(a):
    """[n*128, C] -> [128, n, C] (partition-major, contiguous per partition)."""
    R, C = a.shape
    return np.ascontiguousarray(a.reshape(R // 128, 128, C).transpose(1, 0, 2))


def make_in_maps(x, w_qkv, wo):
    """Host-side sharding: 8 cores = (batch b=c//2, head-group g=c%2)."""
    import ml_dtypes
    bf = ml_dtypes.bfloat16
    x = np.asarray(x, dtype=np.float32)
    w_qkv = np.asarray(w_qkv, dtype=np.float32)
    wo = np.asarray(wo, dtype=np.float32)
    diag = np.triu(np.ones((128, 128), np.float32)).astype(bf)
    in_maps = []
    for c in range(8):
        b, g = c // 2, c % 2
        js = slice(g * JQ, (g + 1) * JQ)
        wq = w_qkv[0:E][js]
        wk = w_qkv[E:2 * E][js]
        wv = w_qkv[2 * E:3 * E][js]
        xT = x[b].T.astype(bf)                                   # [E, L]
        m = {
            "wqkT": np.stack([_shuf(wq.T.astype(bf)), _shuf(wk.T.astype(bf))], 1),
            "wvT": _shuf(wv.T.astype(bf)),
            "woT": _shuf(wo[:, js].T.astype(bf)),
            "diag": diag,
        }
        for cc in range(4):
            m[f"xT{cc}"] = _shuf(xT[:, cc * 512:(cc + 1) * 512])
        in_maps.append(m)
    return in_maps


def _get_nc():
    if "nc" not in _CACHE:
        _CACHE["nc"] = build_nc()
    return _CACHE["nc"]


def kernel(x, mask, w_qkv, wo, _trace=False, _trace_kwargs=None):
    from concourse.bass_utils import run_bass_kernel_spmd

    nc = _get_nc()
    in_maps = make_in_maps(x, w_qkv, wo)
    res = run_bass_kernel_spmd(
        nc, in_maps, core_ids=list(range(8)),
        trace=_trace, **(_trace_kwargs or {}),
    )
    _CACHE["last_results"] = res
    y = np.stack([
        res.results[2 * b]["y"].astype(np.float32)
        + res.results[2 * b + 1]["y"].astype(np.float32)
        for b in range(4)
    ])
    return y.astype(np.float32)


# revision 14
# speedup vs baseline: 1.2942x; 1.2942x over previous
"""Multi-head causal attention (B=4, L=2048, E=1024, H=16) on 8 trn2 NeuronCores.

Sharding: (batch, head-group) grid — core c handles batch b=c//2 and heads
g=c%2 (8 heads each).  Each core computes its heads' QKV projection, causal
attention, and a partial output projection; the host sums the two partials
per batch.

v5: on-chip softmax-denominator broadcast, DMA-lean startup/tail:
  - all matmul operands bf16 (PSUM stays fp32); heads 2m/2m+1 paired in
    disjoint PE row halves for scores; one ACTIVATE exps both heads per
    (window, lk-tile); v carries a ones column so PV also emits the
    denominator row (pv row 64).
  - normalization is now fully on-chip: denominator row -> K=1 matmul
    outer-product broadcast (ones[1,64].T @ denom[1,512]) into a heater
    PSUM tile -> DVE reciprocal [64,512] -> tensor_mul.  Replaces v4's
    two DRAM round trips + 4 DMA issues per window-head (~128 DMA issues,
    ~100us of sync/gpsimd engine time) and shortens the dependency chain
    so the tail collapses.  The chain is emitted as 4 deferred closures
    popped one per later-t so no engine FIFO ever head-of-line blocks.
  - startup: PE warms on a memset tile from ~0.5us (no DMA dependency);
    only xc0+xc1+wvT+wqk are fetched up front (5MB/core; all 8 cores share
    HBM so startup is HBM-bound) on the two HWDGE rings; xc2/xc3 are
    fetched mid-A-phase via sync-ring heater closures; wqkT is split into
    q/k halves so qk_unit(0/4) can start after the q half lands.
  - y partials are written bf16 (halves the 8MB/core output traffic);
    host sums in fp32.
"""

import numpy as np

L = 2048
E = 1024
NH = 8        # heads per core
D = 64
JQ = 512      # feature rows per core (NH*D)
ET = E // 128  # 8 e-tiles
LT = L // 128  # 16 l-tiles

_CACHE = {}
DEBUG_DUMP = False
DEBUG_FULL_SCORES = False  # compute full score rectangles (race-detector aid)


def build_nc():
    import concourse.mybir as mybir
    import concourse.tile as tile
    from concourse import bacc
    from contextlib import ExitStack

    f32 = mybir.dt.float32
    bf16 = mybir.dt.bfloat16
    Exp = mybir.ActivationFunctionType.Exp

    nc = bacc.Bacc("TRN2", target_bir_lowering=False, debug=False)

    # all inputs host-pre-shuffled to SBUF layout (partition dim first)
    xT_d = [nc.declare_dram_parameter(f"xT{c}", [128, ET, 512], bf16, isOutput=False)
            for c in range(4)]
    wqkT_d = nc.declare_dram_parameter("wqkT", [128, 2, ET, JQ], bf16, isOutput=False)
    wvT_d = nc.declare_dram_parameter("wvT", [128, ET, JQ], bf16, isOutput=False)
    woT_d = nc.declare_dram_parameter("woT", [128, 4, E], bf16, isOutput=False)
    diag_d = nc.declare_dram_parameter("diag", [128, 128], bf16, isOutput=False)
    y_d = nc.declare_dram_parameter("y", [L, E], bf16, isOutput=True)

    with ExitStack() as ctx:
        tc = ctx.enter_context(tile.TileContext(nc))

        consts = ctx.enter_context(tc.tile_pool(name="consts", bufs=1))
        warm_sb = consts.tile([128, 128], bf16, name="warm_sb")
        nc.vector.memset(warm_sb, 0.0)
        ones_sb = consts.tile([65, 64], bf16, name="ones_sb")
        nc.vector.memset(ones_sb, 1.0)
        diag_sb = consts.tile([128, 128], bf16, name="diag_sb")
        nc.sync.dma_start(out=diag_sb, in_=diag_d.ap())

        # startup is HBM-bound (all 8 cores fetch at once): land tensors in
        # the order compute consumes them.  sync ring: xc0, xc1; act ring:
        # wvT, then q weights, then k weights.  xc2/xc3 are deferred to
        # A-phase heater closures (sync ring).
        xT_p = ctx.enter_context(tc.tile_pool(name="xT", bufs=1))
        xcs = [xT_p.tile([128, ET, 512], bf16, tag=f"xc{c}", name=f"xc{c}")
               for c in range(4)]
        nc.sync.dma_start(out=xcs[0], in_=xT_d[0].ap())
        nc.sync.dma_start(out=xcs[1], in_=xT_d[1].ap())
        wv_p = ctx.enter_context(tc.tile_pool(name="wv", bufs=1))
        wvT_sb = wv_p.tile([128, ET, JQ], bf16)         # 8KB/part
        nc.scalar.dma_start(out=wvT_sb, in_=wvT_d.ap())
        wqk_p = ctx.enter_context(tc.tile_pool(name="wqk", bufs=1))
        wqkT_sb = wqk_p.tile([128, 2, ET, JQ], bf16)    # 16KB/part
        nc.scalar.dma_start(out=wqkT_sb[:, 0], in_=wqkT_d.ap()[:, 0])
        nc.scalar.dma_start(out=wqkT_sb[:, 1], in_=wqkT_d.ap()[:, 1])

        vaug_p = ctx.enter_context(tc.tile_pool(name="vaug", bufs=1))
        v_aug = vaug_p.tile([128, LT, NH, 65], bf16)    # 16.6KB/part
        nc.vector.memset(v_aug[:, :, :, 64:65], 1.0)

        qk_p = ctx.enter_context(tc.tile_pool(name="qk", bufs=1))
        qT_sb = qk_p.tile([128, 4, L], bf16)            # 16KB/part
        kT_sb = qk_p.tile([128, 4, L], bf16)            # 16KB/part
        ao_p = ctx.enter_context(tc.tile_pool(name="ao", bufs=1))
        aoT_sb = ao_p.tile([128, 4, L], bf16)           # 16KB/part

        sc_pp = ctx.enter_context(tc.tile_pool(name="scpp", bufs=2, space="PSUM"))
        pv_pp = ctx.enter_context(tc.tile_pool(name="pvpp", bufs=2, space="PSUM"))
        hp_pp = ctx.enter_context(tc.tile_pool(name="hppp", bufs=2, space="PSUM"))
        pt_p = ctx.enter_context(tc.tile_pool(name="pt", bufs=12))
        rc_p = ctx.enter_context(tc.tile_pool(name="rc", bufs=4))
        aou_p = ctx.enter_context(tc.tile_pool(name="aou", bufs=6))
        y_p = ctx.enter_context(tc.tile_pool(name="y", bufs=4))

        dq = []  # deferred closures, popped one per attention t

        # ---- projection / outproj units (heater pool; 1 PSUM bank each) ----
        def v_unit(c, i):
            ps = hp_pp.tile([128, 512], f32, tag="hp", name="vps")
            for et in range(ET):
                nc.tensor.matmul(
                    ps,
                    lhsT=xcs[c][:, et, i * 128:(i + 1) * 128],
                    rhs=wvT_sb[:, et, :],
                    start=(et == 0), stop=(et == ET - 1),
                )
            nc.vector.tensor_copy(
                out=v_aug[:, c * 4 + i, :, 0:64],
                in_=ps.rearrange("p (h d) -> p h d", h=NH),
            )

        def qk_unit(jt, c):
            # jt 0..3 = q j-tiles, 4..7 = k j-tiles
            ps = hp_pp.tile([128, 512], f32, tag="hp", name="qkps")
            qk = 0 if jt < 4 else 1
            dst = qT_sb if jt < 4 else kT_sb
            for et in range(ET):
                nc.tensor.matmul(
                    ps,
                    lhsT=wqkT_sb[:, qk, et, (jt % 4) * 128:(jt % 4 + 1) * 128],
                    rhs=xcs[c][:, et, :],
                    start=(et == 0), stop=(et == ET - 1),
                )
            nc.vector.tensor_copy(out=dst[:, jt % 4, c * 512:(c + 1) * 512], in_=ps)

        def op_unit(lt, ec, ring=None):
            ps = hp_pp.tile([128, 512], f32, tag="hp", name="opps")
            for jt in range(4):
                nc.tensor.matmul(
                    ps,
                    lhsT=aoT_sb[:, jt, lt * 128:(lt + 1) * 128],
                    rhs=woT_sb[:, jt, ec * 512:(ec + 1) * 512],
                    start=(jt == 0), stop=(jt == 3),
                )
            yt = y_p.tile([128, 512], bf16, tag="y")
            nc.vector.tensor_copy(out=yt, in_=ps)
            (ring or nc.gpsimd).dma_start(
                out=y_d.ap()[lt * 128:(lt + 1) * 128, ec * 512:(ec + 1) * 512],
                in_=yt,
            )

        # ---- attention ------------------------------------------------
        def pair_unit(m, phase, hw0=(), hw1=(), last=False):
            """Heads (2m, 2m+1); phase 0 = lq windows 0,1; phase 1 = windows 2,3.

            Head A (partitions 0:64) scores land in sc[:, 0:512], head B
            (64:128) in sc[:, 512:1024]; one ACTIVATE exps both.  heaters
            (hw0/hw1 per window) are drained one per t into their own PSUM
            pool, filling PE slack under the exp stream.
            """
            for w, heaters in ((2 * phase, hw0), (2 * phase + 1, hw1)):
                lq0 = w * 512
                nt = 4 * w + 4
                pvA = pv_pp.tile([65, 512], f32, tag="pv", name="pvA")
                pvB = pv_pp.tile([65, 512], f32, tag="pv", name="pvB")
                hq = list(heaters)
                # PV trails scores by TWO tiles so the first PV of this
                # window issues after the previous window's pv readers have
                # released the slots (else it blocks the PE FIFO)
                pend = []

                def emit_pv(p, stop):
                    pe, poff, tt = p
                    for pv, base, h in ((pvA, 0, 2 * m), (pvB, 512, 2 * m + 1)):
                        nc.tensor.matmul(
                            pv[:, poff:512],
                            lhsT=v_aug[:, tt, h, :],
                            rhs=pe[:, base + poff:base + 512],
                            start=(tt == 0), stop=stop,
                            skip_group_check=True,
                        )

                for t in range(nt):
                    off = max(0, t * 128 - lq0)
                    moff = 0 if DEBUG_FULL_SCORES else off
                    sc = sc_pp.tile([128, 1024], f32, tag="sc", name="sc")
                    for po, base in ((0, 0), (64, 512)):
                        nc.tensor.matmul(
                            sc[:, base + moff:base + 512],
                            lhsT=kT_sb[po:po + 64, m, t * 128:(t + 1) * 128],
                            rhs=qT_sb[po:po + 64, m, lq0 + moff:lq0 + 512],
                            start=True, stop=True,
                        )
                    pe = pt_p.tile([128, 1024], bf16, tag="pe", name="pe")
                    nc.scalar.activation(out=pe[:, moff:1024], in_=sc[:, moff:1024],
                                         func=Exp, scale=0.125)
                    if t >= 4 * w:  # diagonal block: zero lk > lq
                        for base in (0, 512):
                            nc.vector.tensor_mul(
                                out=pe[:, base + off:base + off + 128],
                                in0=pe[:, base + off:base + off + 128],
                                in1=diag_sb,
                            )
                    if dq:
                        dq.pop(0)()
                    if hq:
                        hq.pop(0)()
                    if len(pend) == 2:
                        emit_pv(pend.pop(0), stop=False)
                    pend.append((pe, off, t))
                while pend:
                    emit_pv(pend.pop(0), stop=(len(pend) == 0))
                # normalize, fully on-chip: copy pv -> sbuf (bf16), broadcast
                # the denominator row to 64 partitions with a K=1 matmul,
                # reciprocal, multiply.  Emitted as deferred closures popped
                # in LATER windows' t-loops so no FIFO head-of-line blocks on
                # a cross-engine wait.
                aoUs, dens, rcbs = [], [], []
                for pv, nm in ((pvA, "A"), (pvB, "B")):
                    aoU = aou_p.tile([65, 512], bf16, tag="aou", name="aoU" + nm)
                    nc.vector.tensor_copy(out=aoU, in_=pv)
                    aoUs.append(aoU)

                # den tiles come from the PV pool: its slot-reuse WAR is
                # already decoupled from the PE FIFO by the pend-2 delay, so
                # the next window's first PV never stalls on the reciprocal.
                def bcast(aoUs=aoUs, dens=dens):
                    for aoU in aoUs:
                        ps = pv_pp.tile([65, 512], f32, tag="pv", name="denps")
                        nc.tensor.matmul(
                            ps[0:64, :], lhsT=ones_sb[64:65, :],
                            rhs=aoU[64:65, :], start=True, stop=True,
                        )
                        dens.append(ps)

                def recip(dens=dens, rcbs=rcbs):
                    # approx_fast: ~18 correct bits, 5x faster than the
                    # Newton reciprocal (which cost 3.3us/window and stalled
                    # the next window's PV via the pv-slot WAR).  Softmax
                    # denominators are positive and well-conditioned.
                    for ps in dens:
                        rcb = rc_p.tile([64, 512], f32, tag="rcb", name="rcb")
                        nc.vector.reciprocal_approx_fast(out=rcb, in_=ps[0:64, :])
                        rcbs.append(rcb)

                def mk_mul(i, po):
                    def mul(aoUs=aoUs, rcbs=rcbs, po=po, i=i, m=m, lq0=lq0):
                        nc.vector.tensor_mul(
                            out=aoT_sb[po:po + 64, m, lq0:lq0 + 512],
                            in0=aoUs[i][0:64, :], in1=rcbs[i],
                        )
                    return mul

                if last and w == 3:
                    bcast(); recip(); mk_mul(0, 0)(); mk_mul(1, 64)()
                else:
                    dq.append(bcast)
                    dq.append(recip)
                    dq.append(mk_mul(0, 0))
                    dq.append(mk_mul(1, 64))

        # ---- schedule -------------------------------------------------
        # warm the PE pstate on the memset tile from ~0.5us (no DMA dep);
        # the trailing memsets zero the sc slots so diagonal-strip exps
        # never see raw PSUM
        warmA = sc_pp.tile([128, 1024], f32, tag="sc", name="warmA")
        for _ in range(56):
            nc.tensor.matmul(
                warmA[:, 0:128], lhsT=warm_sb, rhs=warm_sb,
                start=True, stop=True, skip_group_check=True,
            )
        nc.vector.memset(warmA, 0.0)
        warmB = sc_pp.tile([128, 1024], f32, tag="sc", name="warmB")
        nc.vector.memset(warmB, 0.0)

        # P0: only pair-0-window-0's prerequisites run serially; everything
        # else overlaps attention as heaters.  v first (needs xc0+wvT, the
        # earliest arrivals), then q/k j-tile 0 as the weight halves land.
        for i in range(4):
            v_unit(0, i)
        qk_unit(0, 0)
        qk_unit(4, 0)

        QK = lambda jt, c: (lambda: qk_unit(jt, c))
        VU = lambda c, i: (lambda: v_unit(c, i))
        OP = lambda lt, ec: (lambda: op_unit(lt, ec))
        DX = lambda c: (lambda: nc.sync.dma_start(out=xcs[c], in_=xT_d[c].ap()))

        # A-phase: windows 0,1; heaters finish the q/k projection and kick
        # off the deferred xc2/xc3 fetches on the idle sync ring.  QK(0,1)/
        # QK(4,1) MUST run in window 0: pair-0-window-1's own scores read
        # their qT/kT columns from t=0 (xc1 lands mid-P0, well before w0-t0).
        pair_unit(0, 0, [QK(0, 1), QK(4, 1), QK(1, 0), QK(5, 0)],
                  [VU(1, 0), VU(1, 1), VU(1, 2), VU(1, 3),
                   QK(1, 1), QK(5, 1), DX(2), DX(3)])
        pair_unit(1, 0, [QK(6, 0), QK(2, 0)], [QK(6, 1), QK(2, 1)])
        pair_unit(2, 0, [QK(7, 0), QK(3, 0)], [QK(7, 1), QK(3, 1)])
        pair_unit(3, 0, [QK(0, 2), QK(0, 3), QK(4, 2)],
                  [QK(4, 3), QK(1, 2), QK(1, 3), QK(5, 2), QK(5, 3)])

        # B-phase: windows 2,3; heaters: v for lk>=1024, remaining q/k,
        # then the output projection as soon as its aoT rows are final
        woT_sb = wqk_p.tile([128, 4, E], bf16, tag="wqkT_sb", name="woT_sb")

        def load_wo():
            nc.gpsimd.dma_start(out=woT_sb, in_=woT_d.ap())

        pair_unit(0, 1, [VU(2, 0), VU(2, 1), VU(2, 2), VU(2, 3)],
                  [VU(3, 0), VU(3, 1), VU(3, 2), VU(3, 3)])
        pair_unit(1, 1, [QK(2, 2), QK(2, 3), QK(6, 2)], [QK(6, 3)])
        pair_unit(2, 1, [QK(3, 2), QK(3, 3), QK(7, 2)],
                  [QK(7, 3), load_wo, OP(0, 0), OP(0, 1), OP(1, 0), OP(1, 1)])
        pair_unit(3, 1, [OP(2, 0), OP(2, 1), OP(3, 0), OP(3, 1),
                         OP(4, 0), OP(4, 1), OP(5, 0), OP(5, 1)],
                  [OP(6, 0), OP(6, 1), OP(7, 0), OP(7, 1),
                   OP(8, 0), OP(8, 1), OP(9, 0), OP(9, 1),
                   OP(10, 0), OP(10, 1), OP(11, 0), OP(11, 1)],
                  last=True)

        # drain deferred closures, then the tail of the output projection
        while dq:
            dq.pop(0)()
        for lt in range(12, LT):
            for ec in range(2):
                op_unit(lt, ec, ring=(nc.sync if ec == 0 else nc.scalar))

        if DEBUG_DUMP:
            dbg_ao = nc.declare_dram_parameter("dbg_ao", [128, 4, L], bf16, isOutput=True)
            dbg_q = nc.declare_dram_parameter("dbg_q", [128, 4, L], bf16, isOutput=True)
            dbg_k = nc.declare_dram_parameter("dbg_k", [128, 4, L], bf16, isOutput=True)
            dbg_v = nc.declare_dram_parameter("dbg_v", [128, LT, NH, 65], bf16, isOutput=True)
            nc.sync.dma_start(out=dbg_ao.ap(), in_=aoT_sb)
            nc.sync.dma_start(out=dbg_q.ap(), in_=qT_sb)
            nc.sync.dma_start(out=dbg_k.ap(), in_=kT_sb)
            nc.sync.dma_start(out=dbg_v.ap(), in_=v_aug)

    nc.compile()
    return nc


def _shuf(a):
    """[n*128, C] -> [128, n, C] (partition-major, contiguous per partition)."""
    R, C = a.shape
    return np.ascontiguousarray(a.reshape(R // 128, 128, C).transpose(1, 0, 2))


def make_in_maps(x, w_qkv, wo):
    """Host-side sharding: 8 cores = (batch b=c//2, head-group g=c%2)."""
    import ml_dtypes
    bf = ml_dtypes.bfloat16
    x = np.asarray(x, dtype=np.float32)
    w_qkv = np.asarray(w_qkv, dtype=np.float32)
    wo = np.asarray(wo, dtype=np.float32)
    diag = np.triu(np.ones((128, 128), np.float32)).astype(bf)
    in_maps = []
    for c in range(8):
        b, g = c // 2, c % 2
        js = slice(g * JQ, (g + 1) * JQ)
        wq = w_qkv[0:E][js]
        wk = w_qkv[E:2 * E][js]
        wv = w_qkv[2 * E:3 * E][js]
        xT = x[b].T.astype(bf)                                   # [E, L]
        m = {
            "wqkT": np.stack([_shuf(wq.T.astype(bf)), _shuf(wk.T.astype(bf))], 1),
            "wvT": _shuf(wv.T.astype(bf)),
            "woT": _shuf(wo[:, js].T.astype(bf)),
            "diag": diag,
        }
        for cc in range(4):
            m[f"xT{cc}"] = _shuf(xT[:, cc * 512:(cc + 1) * 512])
        in_maps.append(m)
    return in_maps


def _get_nc():
    if "nc" not in _CACHE:
        _CACHE["nc"] = build_nc()
    return _CACHE["nc"]


def kernel(x, mask, w_qkv, wo, _trace=False, _trace_kwargs=None):
    from concourse.bass_utils import run_bass_kernel_spmd

    nc = _get_nc()
    in_maps = make_in_maps(x, w_qkv, wo)
    res = run_bass_kernel_spmd(
        nc, in_maps, core_ids=list(range(8)),
        trace=_trace, **(_trace_kwargs or {}),
    )
    _CACHE["last_results"] = res
    y = np.stack([
        res.results[2 * b]["y"].astype(np.float32)
        + res.results[2 * b + 1]["y"].astype(np.float32)
        for b in range(4)
    ])
    return y.astype(np.float32)


# revision 20
# speedup vs baseline: 1.2988x; 1.0035x over previous
"""Multi-head causal attention (B=4, L=2048, E=1024, H=16) on 8 trn2 NeuronCores.

Sharding: (batch, head-group) grid — core c handles batch b=c//2 and heads
g=c%2 (8 heads each).  Each core computes its heads' QKV projection, causal
attention, and a partial output projection; the host sums the two partials
per batch.

v5: on-chip softmax-denominator broadcast, DMA-lean startup/tail:
  - all matmul operands bf16 (PSUM stays fp32); heads 2m/2m+1 paired in
    disjoint PE row halves for scores; one ACTIVATE exps both heads per
    (window, lk-tile); v carries a ones column so PV also emits the
    denominator row (pv row 64).
  - normalization is now fully on-chip: denominator row -> K=1 matmul
    outer-product broadcast (ones[1,64].T @ denom[1,512]) into a heater
    PSUM tile -> DVE reciprocal [64,512] -> tensor_mul.  Replaces v4's
    two DRAM round trips + 4 DMA issues per window-head (~128 DMA issues,
    ~100us of sync/gpsimd engine time) and shortens the dependency chain
    so the tail collapses.  The chain is emitted as 4 deferred closures
    popped one per later-t so no engine FIFO ever head-of-line blocks.
  - startup: PE warms on a memset tile from ~0.5us (no DMA dependency);
    only xc0+xc1+wvT+wqk are fetched up front (5MB/core; all 8 cores share
    HBM so startup is HBM-bound) on the two HWDGE rings; xc2/xc3 are
    fetched mid-A-phase via sync-ring heater closures; wqkT is split into
    q/k halves so qk_unit(0/4) can start after the q half lands.
  - y partials are written bf16 (halves the 8MB/core output traffic);
    host sums in fp32.
"""

import numpy as np

L = 2048
E = 1024
NH = 8        # heads per core
D = 64
JQ = 512      # feature rows per core (NH*D)
ET = E // 128  # 8 e-tiles
LT = L // 128  # 16 l-tiles

_CACHE = {}
DEBUG_DUMP = False
DEBUG_FULL_SCORES = False  # compute full score rectangles (race-detector aid)


def build_nc():
    import concourse.mybir as mybir
    import concourse.tile as tile
    from concourse import bacc
    from contextlib import ExitStack

    f32 = mybir.dt.float32
    bf16 = mybir.dt.bfloat16
    Exp = mybir.ActivationFunctionType.Exp

    nc = bacc.Bacc("TRN2", target_bir_lowering=False, debug=False)

    # all inputs host-pre-shuffled to SBUF layout (partition dim first)
    xT_d = [nc.declare_dram_parameter(f"xT{c}", [128, ET, 512], bf16, isOutput=False)
            for c in range(4)]
    wqkT_d = nc.declare_dram_parameter("wqkT", [128, 2, ET, JQ], bf16, isOutput=False)
    wvT_d = nc.declare_dram_parameter("wvT", [128, ET, JQ], bf16, isOutput=False)
    woT_d = nc.declare_dram_parameter("woT", [128, 4, E], bf16, isOutput=False)
    diag_d = nc.declare_dram_parameter("diag", [128, 128], bf16, isOutput=False)
    y_d = nc.declare_dram_parameter("y", [L, E], bf16, isOutput=True)

    with ExitStack() as ctx:
        tc = ctx.enter_context(tile.TileContext(nc))

        consts = ctx.enter_context(tc.tile_pool(name="consts", bufs=1))
        warm_sb = consts.tile([128, 128], bf16, name="warm_sb")
        nc.vector.memset(warm_sb, 0.0)
        ones_sb = consts.tile([65, 64], bf16, name="ones_sb")
        nc.vector.memset(ones_sb, 1.0)
        diag_sb = consts.tile([128, 128], bf16, name="diag_sb")
        nc.sync.dma_start(out=diag_sb, in_=diag_d.ap())

        # startup is HBM-bound (all 8 cores fetch at once): land tensors in
        # the order compute consumes them.  sync ring: xc0, xc1; act ring:
        # wvT, then q weights, then k weights.  xc2/xc3 are deferred to
        # A-phase heater closures (sync ring).
        xT_p = ctx.enter_context(tc.tile_pool(name="xT", bufs=1))
        xcs = [xT_p.tile([128, ET, 512], bf16, tag=f"xc{c}", name=f"xc{c}")
               for c in range(4)]
        nc.sync.dma_start(out=xcs[0][:, 0:4], in_=xT_d[0].ap()[:, 0:4])
        nc.sync.dma_start(out=xcs[0][:, 4:8], in_=xT_d[0].ap()[:, 4:8])
        nc.sync.dma_start(out=xcs[1], in_=xT_d[1].ap())
        wv_p = ctx.enter_context(tc.tile_pool(name="wv", bufs=1))
        wvT_sb = wv_p.tile([128, ET, JQ], bf16)         # 8KB/part
        nc.scalar.dma_start(out=wvT_sb, in_=wvT_d.ap())
        wqk_p = ctx.enter_context(tc.tile_pool(name="wqk", bufs=1))
        wqkT_sb = wqk_p.tile([128, 2, ET, JQ], bf16)    # 16KB/part
        nc.scalar.dma_start(out=wqkT_sb[:, 0], in_=wqkT_d.ap()[:, 0])
        nc.scalar.dma_start(out=wqkT_sb[:, 1, 0:4], in_=wqkT_d.ap()[:, 1, 0:4])
        nc.scalar.dma_start(out=wqkT_sb[:, 1, 4:8], in_=wqkT_d.ap()[:, 1, 4:8])

        vaug_p = ctx.enter_context(tc.tile_pool(name="vaug", bufs=1))
        v_aug = vaug_p.tile([128, LT, NH, 65], bf16)    # 16.6KB/part
        nc.vector.memset(v_aug[:, :, :, 64:65], 1.0)

        qk_p = ctx.enter_context(tc.tile_pool(name="qk", bufs=1))
        qT_sb = qk_p.tile([128, 4, L], bf16)            # 16KB/part
        kT_sb = qk_p.tile([128, 4, L], bf16)            # 16KB/part
        ao_p = ctx.enter_context(tc.tile_pool(name="ao", bufs=1))
        aoT_sb = ao_p.tile([128, 4, L], bf16)           # 16KB/part

        sc_pp = ctx.enter_context(tc.tile_pool(name="scpp", bufs=2, space="PSUM"))
        pv_pp = ctx.enter_context(tc.tile_pool(name="pvpp", bufs=2, space="PSUM"))
        hp_pp = ctx.enter_context(tc.tile_pool(name="hppp", bufs=2, space="PSUM"))
        pt_p = ctx.enter_context(tc.tile_pool(name="pt", bufs=12))
        rc_p = ctx.enter_context(tc.tile_pool(name="rc", bufs=4))
        aou_p = ctx.enter_context(tc.tile_pool(name="aou", bufs=6))
        y_p = ctx.enter_context(tc.tile_pool(name="y", bufs=4))

        dq = []  # deferred closures, popped one per attention t

        # ---- projection / outproj units (heater pool; 1 PSUM bank each) ----
        def v_unit(c, i):
            ps = hp_pp.tile([128, 512], f32, tag="hp", name="vps")
            for et in range(ET):
                nc.tensor.matmul(
                    ps,
                    lhsT=xcs[c][:, et, i * 128:(i + 1) * 128],
                    rhs=wvT_sb[:, et, :],
                    start=(et == 0), stop=(et == ET - 1),
                )
            nc.vector.tensor_copy(
                out=v_aug[:, c * 4 + i, :, 0:64],
                in_=ps.rearrange("p (h d) -> p h d", h=NH),
            )

        def qk_unit(jt, c):
            # jt 0..3 = q j-tiles, 4..7 = k j-tiles
            ps = hp_pp.tile([128, 512], f32, tag="hp", name="qkps")
            qk = 0 if jt < 4 else 1
            dst = qT_sb if jt < 4 else kT_sb
            for et in range(ET):
                nc.tensor.matmul(
                    ps,
                    lhsT=wqkT_sb[:, qk, et, (jt % 4) * 128:(jt % 4 + 1) * 128],
                    rhs=xcs[c][:, et, :],
                    start=(et == 0), stop=(et == ET - 1),
                )
            nc.vector.tensor_copy(out=dst[:, jt % 4, c * 512:(c + 1) * 512], in_=ps)

        def op_unit(lt, ec, ring=None):
            ps = hp_pp.tile([128, 512], f32, tag="hp", name="opps")
            for jt in range(4):
                nc.tensor.matmul(
                    ps,
                    lhsT=aoT_sb[:, jt, lt * 128:(lt + 1) * 128],
                    rhs=woT_sb[:, jt, ec * 512:(ec + 1) * 512],
                    start=(jt == 0), stop=(jt == 3),
                )
            yt = y_p.tile([128, 512], bf16, tag="y")
            nc.vector.tensor_copy(out=yt, in_=ps)
            (ring or nc.gpsimd).dma_start(
                out=y_d.ap()[lt * 128:(lt + 1) * 128, ec * 512:(ec + 1) * 512],
                in_=yt,
            )

        def v_pair(c, i0):
            # two v units et-interleaved so both start on the first half of
            # a split x-chunk DMA (startup latency)
            pss = [hp_pp.tile([128, 512], f32, tag="hp", name="vps")
                   for _ in range(2)]
            for et in range(ET):
                for u, ps in enumerate(pss):
                    nc.tensor.matmul(
                        ps,
                        lhsT=xcs[c][:, et, (i0 + u) * 128:(i0 + u + 1) * 128],
                        rhs=wvT_sb[:, et, :],
                        start=(et == 0), stop=(et == ET - 1),
                    )
            for u, ps in enumerate(pss):
                nc.vector.tensor_copy(
                    out=v_aug[:, c * 4 + i0 + u, :, 0:64],
                    in_=ps.rearrange("p (h d) -> p h d", h=NH),
                )

        # ---- attention ------------------------------------------------
        def pair_unit(m, phase, hw0=(), hw1=(), last=False):
            """Heads (2m, 2m+1); phase 0 = lq windows 0,1; phase 1 = windows 2,3.

            Head A (partitions 0:64) scores land in sc[:, 0:512], head B
            (64:128) in sc[:, 512:1024]; one ACTIVATE exps both.  heaters
            (hw0/hw1 per window) are drained one per t into their own PSUM
            pool, filling PE slack under the exp stream.
            """
            for w, heaters in ((2 * phase, hw0), (2 * phase + 1, hw1)):
                lq0 = w * 512
                nt = 4 * w + 4
                pvA = pv_pp.tile([65, 512], f32, tag="pv", name="pvA")
                pvB = pv_pp.tile([65, 512], f32, tag="pv", name="pvB")
                hq = list(heaters)
                # PV trails scores by TWO tiles so the first PV of this
                # window issues after the previous window's pv readers have
                # released the slots (else it blocks the PE FIFO)
                pend = []

                def emit_pv(p, stop):
                    pe, poff, tt = p
                    for pv, base, h in ((pvA, 0, 2 * m), (pvB, 512, 2 * m + 1)):
                        nc.tensor.matmul(
                            pv[:, poff:512],
                            lhsT=v_aug[:, tt, h, :],
                            rhs=pe[:, base + poff:base + 512],
                            start=(tt == 0), stop=stop,
                            skip_group_check=True,
                        )

                for t in range(nt):
                    off = max(0, t * 128 - lq0)
                    moff = 0 if DEBUG_FULL_SCORES else off
                    sc = sc_pp.tile([128, 1024], f32, tag="sc", name="sc")
                    for po, base in ((0, 0), (64, 512)):
                        nc.tensor.matmul(
                            sc[:, base + moff:base + 512],
                            lhsT=kT_sb[po:po + 64, m, t * 128:(t + 1) * 128],
                            rhs=qT_sb[po:po + 64, m, lq0 + moff:lq0 + 512],
                            start=True, stop=True,
                        )
                    pe = pt_p.tile([128, 1024], bf16, tag="pe", name="pe")
                    nc.scalar.activation(out=pe[:, moff:1024], in_=sc[:, moff:1024],
                                         func=Exp, scale=0.125)
                    if t >= 4 * w:  # diagonal block: zero lk > lq
                        for base in (0, 512):
                            nc.vector.tensor_mul(
                                out=pe[:, base + off:base + off + 128],
                                in0=pe[:, base + off:base + off + 128],
                                in1=diag_sb,
                            )
                    if dq:
                        dq.pop(0)()
                    if hq:
                        hq.pop(0)()
                    if len(pend) == 2:
                        emit_pv(pend.pop(0), stop=False)
                    pend.append((pe, off, t))
                while pend:
                    emit_pv(pend.pop(0), stop=(len(pend) == 0))
                # normalize, fully on-chip: copy pv -> sbuf (bf16), broadcast
                # the denominator row to 64 partitions with a K=1 matmul,
                # reciprocal, multiply.  Emitted as deferred closures popped
                # in LATER windows' t-loops so no FIFO head-of-line blocks on
                # a cross-engine wait.
                aoUs, dens, rcbs = [], [], []
                for pv, nm in ((pvA, "A"), (pvB, "B")):
                    aoU = aou_p.tile([65, 512], bf16, tag="aou", name="aoU" + nm)
                    nc.vector.tensor_copy(out=aoU, in_=pv)
                    aoUs.append(aoU)

                # den tiles come from the PV pool: its slot-reuse WAR is
                # already decoupled from the PE FIFO by the pend-2 delay, so
                # the next window's first PV never stalls on the reciprocal.
                def bcast(aoUs=aoUs, dens=dens):
                    for aoU in aoUs:
                        ps = pv_pp.tile([65, 512], f32, tag="pv", name="denps")
                        nc.tensor.matmul(
                            ps[0:64, :], lhsT=ones_sb[64:65, :],
                            rhs=aoU[64:65, :], start=True, stop=True,
                        )
                        dens.append(ps)

                def recip(dens=dens, rcbs=rcbs):
                    # approx_fast: ~18 correct bits, 5x faster than the
                    # Newton reciprocal (which cost 3.3us/window and stalled
                    # the next window's PV via the pv-slot WAR).  Softmax
                    # denominators are positive and well-conditioned.
                    for ps in dens:
                        rcb = rc_p.tile([64, 512], f32, tag="rcb", name="rcb")
                        nc.vector.reciprocal_approx_fast(out=rcb, in_=ps[0:64, :])
                        rcbs.append(rcb)

                def mk_mul(i, po):
                    def mul(aoUs=aoUs, rcbs=rcbs, po=po, i=i, m=m, lq0=lq0):
                        nc.vector.tensor_mul(
                            out=aoT_sb[po:po + 64, m, lq0:lq0 + 512],
                            in0=aoUs[i][0:64, :], in1=rcbs[i],
                        )
                    return mul

                if last and w == 3:
                    bcast(); recip(); mk_mul(0, 0)(); mk_mul(1, 64)()
                else:
                    dq.append(bcast)
                    dq.append(recip)
                    dq.append(mk_mul(0, 0))
                    dq.append(mk_mul(1, 64))

        # ---- schedule -------------------------------------------------
        # warm the PE pstate on the memset tile from ~0.5us (no DMA dep);
        # the trailing memsets zero the sc slots so diagonal-strip exps
        # never see raw PSUM
        warmA = sc_pp.tile([128, 1024], f32, tag="sc", name="warmA")
        for _ in range(48):
            nc.tensor.matmul(
                warmA[:, 0:128], lhsT=warm_sb, rhs=warm_sb,
                start=True, stop=True, skip_group_check=True,
            )
        nc.vector.memset(warmA, 0.0)
        warmB = sc_pp.tile([128, 1024], f32, tag="sc", name="warmB")
        nc.vector.memset(warmB, 0.0)

        # P0: only pair-0-window-0's prerequisites run serially; everything
        # else overlaps attention as heaters.  v first (needs xc0+wvT, the
        # earliest arrivals), then q/k j-tile 0 as the weight halves land.
        v_pair(0, 0)
        v_pair(0, 2)
        qk_unit(0, 0)
        qk_unit(4, 0)

        QK = lambda jt, c: (lambda: qk_unit(jt, c))
        VU = lambda c, i: (lambda: v_unit(c, i))
        OP = lambda lt, ec: (lambda: op_unit(lt, ec))
        DX = lambda c: (lambda: nc.sync.dma_start(out=xcs[c], in_=xT_d[c].ap()))

        # A-phase: windows 0,1; heaters finish the q/k projection and kick
        # off the deferred xc2/xc3 fetches on the idle sync ring.  QK(0,1)/
        # QK(4,1) MUST run in window 0: pair-0-window-1's own scores read
        # their qT/kT columns from t=0 (xc1 lands mid-P0, well before w0-t0).
        pair_unit(0, 0, [QK(0, 1), QK(4, 1), QK(1, 0), QK(5, 0)],
                  [VU(1, 0), VU(1, 1), VU(1, 2), VU(1, 3),
                   QK(1, 1), QK(5, 1), DX(2), DX(3)])
        pair_unit(1, 0, [QK(6, 0), QK(2, 0)], [QK(6, 1), QK(2, 1)])
        pair_unit(2, 0, [QK(7, 0), QK(3, 0)], [QK(7, 1), QK(3, 1)])
        pair_unit(3, 0, [QK(0, 2), QK(0, 3), QK(4, 2)],
                  [QK(4, 3), QK(1, 2), QK(1, 3), QK(5, 2), QK(5, 3)])

        # B-phase: windows 2,3; heaters: v for lk>=1024, remaining q/k,
        # then the output projection as soon as its aoT rows are final
        woT_sb = wqk_p.tile([128, 4, E], bf16, tag="wqkT_sb", name="woT_sb")

        def load_wo():
            nc.gpsimd.dma_start(out=woT_sb, in_=woT_d.ap())

        pair_unit(0, 1, [VU(2, 0), VU(2, 1), VU(2, 2), VU(2, 3)],
                  [VU(3, 0), VU(3, 1), VU(3, 2), VU(3, 3)])
        pair_unit(1, 1, [QK(2, 2), QK(2, 3), QK(6, 2)], [QK(6, 3)])
        pair_unit(2, 1, [QK(3, 2), QK(3, 3), QK(7, 2)],
                  [QK(7, 3), load_wo, OP(0, 0), OP(0, 1), OP(1, 0), OP(1, 1)])
        pair_unit(3, 1, [OP(2, 0), OP(2, 1), OP(3, 0), OP(3, 1),
                         OP(4, 0), OP(4, 1), OP(5, 0), OP(5, 1)],
                  [OP(6, 0), OP(6, 1), OP(7, 0), OP(7, 1),
                   OP(8, 0), OP(8, 1), OP(9, 0), OP(9, 1),
                   OP(10, 0), OP(10, 1), OP(11, 0), OP(11, 1)],
                  last=True)

        # drain deferred closures, then the tail of the output projection
        while dq:
            dq.pop(0)()
        for lt in range(12, LT):
            for ec in range(2):
                op_unit(lt, ec, ring=(nc.sync if ec == 0 else nc.scalar))

        if DEBUG_DUMP:
            dbg_ao = nc.declare_dram_parameter("dbg_ao", [128, 4, L], bf16, isOutput=True)
            dbg_q = nc.declare_dram_parameter("dbg_q", [128, 4, L], bf16, isOutput=True)
            dbg_k = nc.declare_dram_parameter("dbg_k", [128, 4, L], bf16, isOutput=True)
            dbg_v = nc.declare_dram_parameter("dbg_v", [128, LT, NH, 65], bf16, isOutput=True)
            nc.sync.dma_start(out=dbg_ao.ap(), in_=aoT_sb)
            nc.sync.dma_start(out=dbg_q.ap(), in_=qT_sb)
            nc.sync.dma_start(out=dbg_k.ap(), in_=kT_sb)
            nc.sync.dma_start(out=dbg_v.ap(), in_=v_aug)

    nc.compile()
    return nc


def _shuf(a):
    """[n*128, C] -> [128, n, C] (partition-major, contiguous per partition)."""
    R, C = a.shape
    return np.ascontiguousarray(a.reshape(R // 128, 128, C).transpose(1, 0, 2))


def make_in_maps(x, w_qkv, wo):
    """Host-side sharding: 8 cores = (batch b=c//2, head-group g=c%2)."""
    import ml_dtypes
    bf = ml_dtypes.bfloat16
    x = np.asarray(x, dtype=np.float32)
    w_qkv = np.asarray(w_qkv, dtype=np.float32)
    wo = np.asarray(wo, dtype=np.float32)
    diag = np.triu(np.ones((128, 128), np.float32)).astype(bf)
    in_maps = []
    for c in range(8):
        b, g = c // 2, c % 2
        js = slice(g * JQ, (g + 1) * JQ)
        wq = w_qkv[0:E][js]
        wk = w_qkv[E:2 * E][js]
        wv = w_qkv[2 * E:3 * E][js]
        xT = x[b].T.astype(bf)                                   # [E, L]
        m = {
            "wqkT": np.stack([_shuf(wq.T.astype(bf)), _shuf(wk.T.astype(bf))], 1),
            "wvT": _shuf(wv.T.astype(bf)),
            "woT": _shuf(wo[:, js].T.astype(bf)),
            "diag": diag,
        }
        for cc in range(4):
            m[f"xT{cc}"] = _shuf(xT[:, cc * 512:(cc + 1) * 512])
        in_maps.append(m)
    return in_maps


def _get_nc():
    if "nc" not in _CACHE:
        _CACHE["nc"] = build_nc()
    return _CACHE["nc"]


def kernel(x, mask, w_qkv, wo, _trace=False, _trace_kwargs=None):
    from concourse.bass_utils import run_bass_kernel_spmd

    nc = _get_nc()
    in_maps = make_in_maps(x, w_qkv, wo)
    res = run_bass_kernel_spmd(
        nc, in_maps, core_ids=list(range(8)),
        trace=_trace, **(_trace_kwargs or {}),
    )
    _CACHE["last_results"] = res
    y = np.stack([
        res.results[2 * b]["y"].astype(np.float32)
        + res.results[2 * b + 1]["y"].astype(np.float32)
        for b in range(4)
    ])
    return y.astype(np.float32)


# revision 22
# speedup vs baseline: 1.4431x; 1.1111x over previous
"""Multi-head causal attention (B=4, L=2048, E=1024, H=16) on 8 trn2 NeuronCores.

Sharding: (batch, head-group) grid — core c handles batch b=c//2 and heads
g=c%2 (8 heads each).  Each core computes its heads' QKV projection, causal
attention, and a partial output projection; the host sums the two partials
per batch.

v5: on-chip softmax-denominator broadcast, DMA-lean startup/tail:
  - all matmul operands bf16 (PSUM stays fp32); heads 2m/2m+1 paired in
    disjoint PE row halves for scores; one ACTIVATE exps both heads per
    (window, lk-tile); v carries a ones column so PV also emits the
    denominator row (pv row 64).
  - normalization is now fully on-chip: denominator row -> K=1 matmul
    outer-product broadcast (ones[1,64].T @ denom[1,512]) into a heater
    PSUM tile -> DVE reciprocal [64,512] -> tensor_mul.  Replaces v4's
    two DRAM round trips + 4 DMA issues per window-head (~128 DMA issues,
    ~100us of sync/gpsimd engine time) and shortens the dependency chain
    so the tail collapses.  The chain is emitted as 4 deferred closures
    popped one per later-t so no engine FIFO ever head-of-line blocks.
  - startup: PE warms on a memset tile from ~0.5us (no DMA dependency);
    only xc0+xc1+wvT+wqk are fetched up front (5MB/core; all 8 cores share
    HBM so startup is HBM-bound) on the two HWDGE rings; xc2/xc3 are
    fetched mid-A-phase via sync-ring heater closures; wqkT is split into
    q/k halves so qk_unit(0/4) can start after the q half lands.
  - y partials are written bf16 (halves the 8MB/core output traffic);
    host sums in fp32.
"""

import numpy as np

L = 2048
E = 1024
NH = 8        # heads per core
D = 64
JQ = 512      # feature rows per core (NH*D)
ET = E // 128  # 8 e-tiles
LT = L // 128  # 16 l-tiles

_CACHE = {}
DEBUG_DUMP = False
DEBUG_FULL_SCORES = False  # compute full score rectangles (race-detector aid)


def build_nc():
    import concourse.mybir as mybir
    import concourse.tile as tile
    from concourse import bacc
    from contextlib import ExitStack

    f32 = mybir.dt.float32
    bf16 = mybir.dt.bfloat16
    Exp = mybir.ActivationFunctionType.Exp

    nc = bacc.Bacc("TRN2", target_bir_lowering=False, debug=False)

    # all inputs host-pre-shuffled to SBUF layout (partition dim first)
    xT_d = [nc.declare_dram_parameter(f"xT{c}", [128, ET, 512], bf16, isOutput=False)
            for c in range(4)]
    wqkT_d = nc.declare_dram_parameter("wqkT", [128, 2, ET, JQ], bf16, isOutput=False)
    wvT_d = nc.declare_dram_parameter("wvT", [128, ET, JQ], bf16, isOutput=False)
    woT_d = nc.declare_dram_parameter("woT", [128, 4, E], bf16, isOutput=False)
    diag_d = nc.declare_dram_parameter("diag", [128, 256], bf16, isOutput=False)
    y_d = nc.declare_dram_parameter("y", [L, E], bf16, isOutput=True)

    with ExitStack() as ctx:
        tc = ctx.enter_context(tile.TileContext(nc))

        consts = ctx.enter_context(tc.tile_pool(name="consts", bufs=1))
        warm_sb = consts.tile([128, 128], bf16, name="warm_sb")
        nc.vector.memset(warm_sb, 0.0)
        diag_sb = consts.tile([128, 256], bf16, name="diag_sb")
        nc.sync.dma_start(out=diag_sb, in_=diag_d.ap())
        ident_sb = diag_sb[:, 128:256]

        # startup is HBM-bound (all 8 cores fetch at once): land tensors in
        # the order compute consumes them.  sync ring: xc0, xc1; act ring:
        # wvT, then q weights, then k weights.  xc2/xc3 are deferred to
        # A-phase heater closures (sync ring).
        xT_p = ctx.enter_context(tc.tile_pool(name="xT", bufs=1))
        xcs = [xT_p.tile([128, ET, 512], bf16, tag=f"xc{c}", name=f"xc{c}")
               for c in range(4)]
        nc.sync.dma_start(out=xcs[0][:, 0:4], in_=xT_d[0].ap()[:, 0:4])
        nc.sync.dma_start(out=xcs[0][:, 4:8], in_=xT_d[0].ap()[:, 4:8])
        nc.sync.dma_start(out=xcs[1], in_=xT_d[1].ap())
        wv_p = ctx.enter_context(tc.tile_pool(name="wv", bufs=1))
        wvT_sb = wv_p.tile([128, ET, JQ], bf16)         # 8KB/part
        nc.scalar.dma_start(out=wvT_sb, in_=wvT_d.ap())
        wqk_p = ctx.enter_context(tc.tile_pool(name="wqk", bufs=1))
        wqkT_sb = wqk_p.tile([128, 2, ET, JQ], bf16)    # 16KB/part
        nc.scalar.dma_start(out=wqkT_sb[:, 0], in_=wqkT_d.ap()[:, 0])
        nc.scalar.dma_start(out=wqkT_sb[:, 1, 0:4], in_=wqkT_d.ap()[:, 1, 0:4])
        nc.scalar.dma_start(out=wqkT_sb[:, 1, 4:8], in_=wqkT_d.ap()[:, 1, 4:8])

        vaug_p = ctx.enter_context(tc.tile_pool(name="vaug", bufs=1))
        v_aug = vaug_p.tile([128, LT, NH, 65], bf16)    # 16.6KB/part
        nc.vector.memset(v_aug[:, :, :, 64:65], 1.0)

        qk_p = ctx.enter_context(tc.tile_pool(name="qk", bufs=1))
        qT_sb = qk_p.tile([128, 4, L], bf16)            # 16KB/part
        kT_sb = qk_p.tile([128, 4, L], bf16)            # 16KB/part
        ao_p = ctx.enter_context(tc.tile_pool(name="ao", bufs=1))
        aoT_sb = ao_p.tile([128, 4, L], bf16)           # 16KB/part

        sc_pp = ctx.enter_context(tc.tile_pool(name="scpp", bufs=2, space="PSUM"))
        pv_pp = ctx.enter_context(tc.tile_pool(name="pvpp", bufs=2, space="PSUM"))
        hp_pp = ctx.enter_context(tc.tile_pool(name="hppp", bufs=2, space="PSUM"))
        pt_p = ctx.enter_context(tc.tile_pool(name="pt", bufs=22))
        rc_p = ctx.enter_context(tc.tile_pool(name="rc", bufs=4))
        aon_p = ctx.enter_context(tc.tile_pool(name="aon", bufs=4))
        y_p = ctx.enter_context(tc.tile_pool(name="y", bufs=4))

        dq = []  # deferred closures, popped one per attention t

        # ---- projection / outproj units (heater pool; 1 PSUM bank each) ----
        def v_unit(c, i):
            ps = hp_pp.tile([128, 512], f32, tag="hp", name="vps")
            for et in range(ET):
                nc.tensor.matmul(
                    ps,
                    lhsT=xcs[c][:, et, i * 128:(i + 1) * 128],
                    rhs=wvT_sb[:, et, :],
                    start=(et == 0), stop=(et == ET - 1),
                )
            nc.vector.tensor_copy(
                out=v_aug[:, c * 4 + i, :, 0:64],
                in_=ps.rearrange("p (h d) -> p h d", h=NH),
            )

        def qk_unit(jt, c):
            # jt 0..3 = q j-tiles, 4..7 = k j-tiles
            ps = hp_pp.tile([128, 512], f32, tag="hp", name="qkps")
            qk = 0 if jt < 4 else 1
            dst = qT_sb if jt < 4 else kT_sb
            for et in range(ET):
                nc.tensor.matmul(
                    ps,
                    lhsT=wqkT_sb[:, qk, et, (jt % 4) * 128:(jt % 4 + 1) * 128],
                    rhs=xcs[c][:, et, :],
                    start=(et == 0), stop=(et == ET - 1),
                )
            nc.vector.tensor_copy(out=dst[:, jt % 4, c * 512:(c + 1) * 512], in_=ps)

        def op_unit(lt, ec, ring=None):
            ps = hp_pp.tile([128, 512], f32, tag="hp", name="opps")
            for jt in range(4):
                nc.tensor.matmul(
                    ps,
                    lhsT=aoT_sb[:, jt, lt * 128:(lt + 1) * 128],
                    rhs=woT_sb[:, jt, ec * 512:(ec + 1) * 512],
                    start=(jt == 0), stop=(jt == 3),
                )
            yt = y_p.tile([128, 512], bf16, tag="y")
            nc.vector.tensor_copy(out=yt, in_=ps)
            (ring or nc.gpsimd).dma_start(
                out=y_d.ap()[lt * 128:(lt + 1) * 128, ec * 512:(ec + 1) * 512],
                in_=yt,
            )

        def v_pair(c, i0):
            # two v units et-interleaved so both start on the first half of
            # a split x-chunk DMA (startup latency)
            pss = [hp_pp.tile([128, 512], f32, tag="hp", name="vps")
                   for _ in range(2)]
            for et in range(ET):
                for u, ps in enumerate(pss):
                    nc.tensor.matmul(
                        ps,
                        lhsT=xcs[c][:, et, (i0 + u) * 128:(i0 + u + 1) * 128],
                        rhs=wvT_sb[:, et, :],
                        start=(et == 0), stop=(et == ET - 1),
                    )
            for u, ps in enumerate(pss):
                nc.vector.tensor_copy(
                    out=v_aug[:, c * 4 + i0 + u, :, 0:64],
                    in_=ps.rearrange("p (h d) -> p h d", h=NH),
                )

        # ---- attention ------------------------------------------------
        def pair_unit(m, phase, hw0=(), hw1=(), last=False):
            """Heads (2m, 2m+1); phase 0 = lq windows 0,1; phase 1 = windows 2,3.

            Head A (partitions 0:64) scores land in sc[:, 0:512], head B
            (64:128) in sc[:, 512:1024]; one ACTIVATE exps both.  heaters
            (hw0/hw1 per window) are drained one per t into their own PSUM
            pool, filling PE slack under the exp stream.
            """
            for w, heaters in ((2 * phase, hw0), (2 * phase + 1, hw1)):
                lq0 = w * 512
                nt = 4 * w + 4
                # chunk-major transposed PV: the window's exp'd tiles are
                # saved and the PV runs as deferred per-chunk bursts (popped
                # 2/t in later windows).  Each lq-chunk j is one PSUM
                # accumulation group [128, 65] at column 128j; groups in a
                # bank are strictly sequential (PSUM zero regions are
                # bank-granular: an interleaved group's start would wipe its
                # siblings' partial sums).  pe chunk is the stationary
                # operand (M=128 full PE width); streaming is 65 cols vs the
                # v4 orientation's 512 — PV cycles halve.  Denominators land
                # per-partition at pv[:, 128j+64]: reciprocal_approx_fast +
                # tensor_scalar muls, then 8 PE transposes + one cast copy
                # rebuild aoT.
                pes = []
                hq = list(heaters)

                for t in range(nt):
                    off = max(0, t * 128 - lq0)
                    moff = 0 if DEBUG_FULL_SCORES else off
                    sc = sc_pp.tile([128, 1024], f32, tag="sc", name="sc")
                    for po, base in ((0, 0), (64, 512)):
                        nc.tensor.matmul(
                            sc[:, base + moff:base + 512],
                            lhsT=kT_sb[po:po + 64, m, t * 128:(t + 1) * 128],
                            rhs=qT_sb[po:po + 64, m, lq0 + moff:lq0 + 512],
                            start=True, stop=True,
                        )
                    pe = pt_p.tile([128, 1024], bf16, tag="pe", name="pe")
                    nc.scalar.activation(out=pe[:, moff:1024], in_=sc[:, moff:1024],
                                         func=Exp, scale=0.125)
                    if t >= 4 * w:  # diagonal block: zero lk > lq
                        for base in (0, 512):
                            nc.vector.tensor_mul(
                                out=pe[:, base + off:base + off + 128],
                                in0=pe[:, base + off:base + off + 128],
                                in1=diag_sb[:, 0:128],
                            )
                    if dq:
                        dq.pop(0)()
                    if dq:
                        dq.pop(0)()
                    if hq:
                        hq.pop(0)()
                    pes.append(pe)

                pvs, rcps, aons, tps = [], [], [], []

                def mk_pvc(j, pes=pes, pvs=pvs, w=w, m=m):
                    def pvc():
                        if j == 0:
                            pvs.append(pv_pp.tile([128, 512], f32, tag="pv",
                                                  name="pvA"))
                            pvs.append(pv_pp.tile([128, 512], f32, tag="pv",
                                                  name="pvB"))
                        for i, base in ((0, 0), (1, 512)):
                            pv, h = pvs[i], 2 * m + i
                            for tt in range(4 * w + j + 1):
                                nc.tensor.matmul(
                                    pv[:, 128 * j:128 * j + 65],
                                    lhsT=pes[tt][:, base + 128 * j:
                                                 base + 128 * (j + 1)],
                                    rhs=v_aug[:, tt, h, :],
                                    start=(tt == 0), stop=(tt == 4 * w + j),
                                    skip_group_check=True,
                                )
                    return pvc

                def recips(rcps=rcps, pvs=pvs):
                    for pv in pvs:
                        rcp4 = rc_p.tile([128, 4], f32, tag="rcp", name="rcp4")
                        nc.vector.reciprocal_approx_fast(
                            out=rcp4.rearrange("p (j o) -> p j o", o=1),
                            in_=pv.rearrange("p (j c) -> p j c", c=128)[:, :, 64:65],
                        )
                        rcps.append(rcp4)

                def mk_muls(i):
                    def muls(rcps=rcps, aons=aons, i=i, pvs=pvs):
                        pv = pvs[i]
                        aon = aon_p.tile([128, 256], bf16, tag="aon", name="aon")
                        for j in range(4):
                            nc.vector.tensor_scalar_mul(
                                out=aon[:, 64 * j:64 * j + 64],
                                in0=pv[:, 128 * j:128 * j + 64],
                                scalar1=rcps[i][:, j:j + 1],
                            )
                        aons.append(aon)
                    return muls

                def trans(aons=aons, tps=tps):
                    tp = hp_pp.tile([128, 512], bf16, tag="hp", name="tp")
                    for i, po in ((0, 0), (1, 64)):
                        for j in range(4):
                            nc.tensor.matmul(
                                tp[po:po + 64, 128 * j:128 * (j + 1)],
                                lhsT=aons[i][:, 64 * j:64 * j + 64],
                                rhs=ident_sb,
                                start=True, stop=True, is_transpose=True,
                                skip_group_check=True,
                            )
                    tps.append(tp)

                def fold(tps=tps, m=m, lq0=lq0):
                    nc.vector.tensor_copy(
                        out=aoT_sb[:, m, lq0:lq0 + 512], in_=tps[0])

                chain = [mk_pvc(0), mk_pvc(1), mk_pvc(2), mk_pvc(3),
                         recips, mk_muls(0), mk_muls(1), trans, fold]
                if last and w == 3:
                    for f in chain:
                        f()
                else:
                    dq.extend(chain)

        # ---- schedule -------------------------------------------------
        # warm the PE pstate on the memset tile from ~0.5us (no DMA dep);
        # the trailing memsets zero the sc slots so diagonal-strip exps
        # never see raw PSUM
        warmA = sc_pp.tile([128, 1024], f32, tag="sc", name="warmA")
        for _ in range(48):
            nc.tensor.matmul(
                warmA[:, 0:128], lhsT=warm_sb, rhs=warm_sb,
                start=True, stop=True, skip_group_check=True,
            )
        nc.vector.memset(warmA, 0.0)
        warmB = sc_pp.tile([128, 1024], f32, tag="sc", name="warmB")
        nc.vector.memset(warmB, 0.0)

        # P0: only pair-0-window-0's prerequisites run serially; everything
        # else overlaps attention as heaters.  v first (needs xc0+wvT, the
        # earliest arrivals), then q/k j-tile 0 as the weight halves land.
        v_pair(0, 0)
        v_pair(0, 2)
        qk_unit(0, 0)
        qk_unit(4, 0)

        QK = lambda jt, c: (lambda: qk_unit(jt, c))
        VU = lambda c, i: (lambda: v_unit(c, i))
        OP = lambda lt, ec: (lambda: op_unit(lt, ec))
        DX = lambda c: (lambda: nc.sync.dma_start(out=xcs[c], in_=xT_d[c].ap()))

        # A-phase: windows 0,1; heaters finish the q/k projection and kick
        # off the deferred xc2/xc3 fetches on the idle sync ring.  QK(0,1)/
        # QK(4,1) MUST run in window 0: pair-0-window-1's own scores read
        # their qT/kT columns from t=0 (xc1 lands mid-P0, well before w0-t0).
        pair_unit(0, 0, [QK(0, 1), QK(4, 1), QK(1, 0), QK(5, 0)],
                  [VU(1, 0), VU(1, 1), VU(1, 2), VU(1, 3),
                   QK(1, 1), QK(5, 1), DX(2), DX(3)])
        pair_unit(1, 0, [QK(6, 0), QK(2, 0)], [QK(6, 1), QK(2, 1)])
        pair_unit(2, 0, [QK(7, 0), QK(3, 0)], [QK(7, 1), QK(3, 1)])
        pair_unit(3, 0, [QK(0, 2), QK(0, 3), QK(4, 2)],
                  [QK(4, 3), QK(1, 2), QK(1, 3), QK(5, 2), QK(5, 3)])

        # B-phase: windows 2,3; heaters: v for lk>=1024, remaining q/k,
        # then the output projection as soon as its aoT rows are final
        woT_sb = wqk_p.tile([128, 4, E], bf16, tag="wqkT_sb", name="woT_sb")

        def load_wo():
            nc.gpsimd.dma_start(out=woT_sb, in_=woT_d.ap())

        pair_unit(0, 1, [VU(2, 0), VU(2, 1), VU(2, 2), VU(2, 3)],
                  [VU(3, 0), VU(3, 1), VU(3, 2), VU(3, 3)])
        pair_unit(1, 1, [QK(2, 2), QK(2, 3), QK(6, 2)], [QK(6, 3)])
        pair_unit(2, 1, [QK(3, 2), QK(3, 3), QK(7, 2)],
                  [QK(7, 3), load_wo, OP(0, 0), OP(0, 1), OP(1, 0), OP(1, 1)])
        pair_unit(3, 1, [OP(2, 0), OP(2, 1), OP(3, 0), OP(3, 1),
                         OP(4, 0), OP(4, 1), OP(5, 0), OP(5, 1)],
                  [OP(6, 0), OP(6, 1), OP(7, 0), OP(7, 1),
                   OP(8, 0), OP(8, 1), OP(9, 0), OP(9, 1),
                   OP(10, 0), OP(10, 1), OP(11, 0), OP(11, 1)],
                  last=True)

        # drain deferred closures, then the tail of the output projection
        while dq:
            dq.pop(0)()
        for lt in range(12, LT):
            for ec in range(2):
                op_unit(lt, ec, ring=(nc.sync if ec == 0 else nc.scalar))

        if DEBUG_DUMP:
            dbg_ao = nc.declare_dram_parameter("dbg_ao", [128, 4, L], bf16, isOutput=True)
            dbg_q = nc.declare_dram_parameter("dbg_q", [128, 4, L], bf16, isOutput=True)
            dbg_k = nc.declare_dram_parameter("dbg_k", [128, 4, L], bf16, isOutput=True)
            dbg_v = nc.declare_dram_parameter("dbg_v", [128, LT, NH, 65], bf16, isOutput=True)
            nc.sync.dma_start(out=dbg_ao.ap(), in_=aoT_sb)
            nc.sync.dma_start(out=dbg_q.ap(), in_=qT_sb)
            nc.sync.dma_start(out=dbg_k.ap(), in_=kT_sb)
            nc.sync.dma_start(out=dbg_v.ap(), in_=v_aug)

    nc.compile()
    return nc


def _shuf(a):
    """[n*128, C] -> [128, n, C] (partition-major, contiguous per partition)."""
    R, C = a.shape
    return np.ascontiguousarray(a.reshape(R // 128, 128, C).transpose(1, 0, 2))


def make_in_maps(x, w_qkv, wo):
    """Host-side sharding: 8 cores = (batch b=c//2, head-group g=c%2)."""
    import ml_dtypes
    bf = ml_dtypes.bfloat16
    x = np.asarray(x, dtype=np.float32)
    w_qkv = np.asarray(w_qkv, dtype=np.float32)
    wo = np.asarray(wo, dtype=np.float32)
    diag = np.concatenate(
        [np.triu(np.ones((128, 128), np.float32)), np.eye(128, dtype=np.float32)],
        axis=1,
    ).astype(bf)
    in_maps = []
    for c in range(8):
        b, g = c // 2, c % 2
        js = slice(g * JQ, (g + 1) * JQ)
        wq = w_qkv[0:E][js]
        wk = w_qkv[E:2 * E][js]
        wv = w_qkv[2 * E:3 * E][js]
        xT = x[b].T.astype(bf)                                   # [E, L]
        m = {
            "wqkT": np.stack([_shuf(wq.T.astype(bf)), _shuf(wk.T.astype(bf))], 1),
            "wvT": _shuf(wv.T.astype(bf)),
            "woT": _shuf(wo[:, js].T.astype(bf)),
            "diag": diag,
        }
        for cc in range(4):
            m[f"xT{cc}"] = _shuf(xT[:, cc * 512:(cc + 1) * 512])
        in_maps.append(m)
    return in_maps


def _get_nc():
    if "nc" not in _CACHE:
        _CACHE["nc"] = build_nc()
    return _CACHE["nc"]


def kernel(x, mask, w_qkv, wo, _trace=False, _trace_kwargs=None):
    from concourse.bass_utils import run_bass_kernel_spmd

    nc = _get_nc()
    in_maps = make_in_maps(x, w_qkv, wo)
    res = run_bass_kernel_spmd(
        nc, in_maps, core_ids=list(range(8)),
        trace=_trace, **(_trace_kwargs or {}),
    )
    _CACHE["last_results"] = res
    y = np.stack([
        res.results[2 * b]["y"].astype(np.float32)
        + res.results[2 * b + 1]["y"].astype(np.float32)
        for b in range(4)
    ])
    return y.astype(np.float32)


# revision 24
# speedup vs baseline: 1.4674x; 1.0168x over previous
"""Multi-head causal attention (B=4, L=2048, E=1024, H=16) on 8 trn2 NeuronCores.

Sharding: (batch, head-group) grid — core c handles batch b=c//2 and heads
g=c%2 (8 heads each).  Each core computes its heads' QKV projection, causal
attention, and a partial output projection; the host sums the two partials
per batch.

v5: on-chip softmax-denominator broadcast, DMA-lean startup/tail:
  - all matmul operands bf16 (PSUM stays fp32); heads 2m/2m+1 paired in
    disjoint PE row halves for scores; one ACTIVATE exps both heads per
    (window, lk-tile); v carries a ones column so PV also emits the
    denominator row (pv row 64).
  - normalization is now fully on-chip: denominator row -> K=1 matmul
    outer-product broadcast (ones[1,64].T @ denom[1,512]) into a heater
    PSUM tile -> DVE reciprocal [64,512] -> tensor_mul.  Replaces v4's
    two DRAM round trips + 4 DMA issues per window-head (~128 DMA issues,
    ~100us of sync/gpsimd engine time) and shortens the dependency chain
    so the tail collapses.  The chain is emitted as 4 deferred closures
    popped one per later-t so no engine FIFO ever head-of-line blocks.
  - startup: PE warms on a memset tile from ~0.5us (no DMA dependency);
    only xc0+xc1+wvT+wqk are fetched up front (5MB/core; all 8 cores share
    HBM so startup is HBM-bound) on the two HWDGE rings; xc2/xc3 are
    fetched mid-A-phase via sync-ring heater closures; wqkT is split into
    q/k halves so qk_unit(0/4) can start after the q half lands.
  - y partials are written bf16 (halves the 8MB/core output traffic);
    host sums in fp32.
"""

import numpy as np

L = 2048
E = 1024
NH = 8        # heads per core
D = 64
JQ = 512      # feature rows per core (NH*D)
ET = E // 128  # 8 e-tiles
LT = L // 128  # 16 l-tiles

_CACHE = {}
DEBUG_DUMP = False
DEBUG_FULL_SCORES = False  # compute full score rectangles (race-detector aid)


def build_nc():
    import concourse.mybir as mybir
    import concourse.tile as tile
    from concourse import bacc
    from contextlib import ExitStack

    f32 = mybir.dt.float32
    bf16 = mybir.dt.bfloat16
    Exp = mybir.ActivationFunctionType.Exp

    nc = bacc.Bacc("TRN2", target_bir_lowering=False, debug=False)

    # all inputs host-pre-shuffled to SBUF layout (partition dim first)
    xT_d = [nc.declare_dram_parameter(f"xT{c}", [128, ET, 512], bf16, isOutput=False)
            for c in range(4)]
    wqkT_d = nc.declare_dram_parameter("wqkT", [128, 2, ET, JQ], bf16, isOutput=False)
    wvT_d = nc.declare_dram_parameter("wvT", [128, ET, JQ], bf16, isOutput=False)
    woT_d = nc.declare_dram_parameter("woT", [128, 4, E], bf16, isOutput=False)
    diag_d = nc.declare_dram_parameter("diag", [128, 256], bf16, isOutput=False)
    y_d = nc.declare_dram_parameter("y", [L, E], bf16, isOutput=True)

    with ExitStack() as ctx:
        tc = ctx.enter_context(tile.TileContext(nc))

        consts = ctx.enter_context(tc.tile_pool(name="consts", bufs=1))
        warm_sb = consts.tile([128, 128], bf16, name="warm_sb")
        nc.vector.memset(warm_sb, 0.0)
        diag_sb = consts.tile([128, 256], bf16, name="diag_sb")
        ident_sb = diag_sb[:, 128:256]

        # startup is HBM-bound (all 8 cores fetch at once): land tensors in
        # the order compute consumes them.  sync ring: xc0, xc1; act ring:
        # wvT, then q weights, then k weights.  xc2/xc3 are deferred to
        # A-phase heater closures (sync ring).
        xT_p = ctx.enter_context(tc.tile_pool(name="xT", bufs=1))
        xcs = [xT_p.tile([128, ET, 512], bf16, tag=f"xc{c}", name=f"xc{c}")
               for c in range(4)]
        nc.sync.dma_start(out=xcs[0][:, 0:4], in_=xT_d[0].ap()[:, 0:4])
        nc.sync.dma_start(out=xcs[0][:, 4:8], in_=xT_d[0].ap()[:, 4:8])
        nc.sync.dma_start(out=xcs[1], in_=xT_d[1].ap())
        # diag/identity aren't consumed until the first window's mask
        # (~22us), so they ride after the critical x chunks
        nc.sync.dma_start(out=diag_sb, in_=diag_d.ap())
        wv_p = ctx.enter_context(tc.tile_pool(name="wv", bufs=1))
        wvT_sb = wv_p.tile([128, ET, JQ], bf16)         # 8KB/part
        nc.scalar.dma_start(out=wvT_sb, in_=wvT_d.ap())
        wqk_p = ctx.enter_context(tc.tile_pool(name="wqk", bufs=1))
        wqkT_sb = wqk_p.tile([128, 2, ET, JQ], bf16)    # 16KB/part
        nc.scalar.dma_start(out=wqkT_sb[:, 0], in_=wqkT_d.ap()[:, 0])
        nc.scalar.dma_start(out=wqkT_sb[:, 1, 0:4], in_=wqkT_d.ap()[:, 1, 0:4])
        nc.scalar.dma_start(out=wqkT_sb[:, 1, 4:8], in_=wqkT_d.ap()[:, 1, 4:8])

        vaug_p = ctx.enter_context(tc.tile_pool(name="vaug", bufs=1))
        v_aug = vaug_p.tile([128, LT, NH, 65], bf16)    # 16.6KB/part
        nc.vector.memset(v_aug[:, :, :, 64:65], 1.0)

        qk_p = ctx.enter_context(tc.tile_pool(name="qk", bufs=1))
        qT_sb = qk_p.tile([128, 4, L], bf16)            # 16KB/part
        kT_sb = qk_p.tile([128, 4, L], bf16)            # 16KB/part
        ao_p = ctx.enter_context(tc.tile_pool(name="ao", bufs=1))
        aoT_sb = ao_p.tile([128, 4, L], bf16)           # 16KB/part

        sc_pp = ctx.enter_context(tc.tile_pool(name="scpp", bufs=2, space="PSUM"))
        pv_pp = ctx.enter_context(tc.tile_pool(name="pvpp", bufs=2, space="PSUM"))
        hp_pp = ctx.enter_context(tc.tile_pool(name="hppp", bufs=2, space="PSUM"))
        pt_p = ctx.enter_context(tc.tile_pool(name="pt", bufs=22))
        rc_p = ctx.enter_context(tc.tile_pool(name="rc", bufs=4))
        aon_p = ctx.enter_context(tc.tile_pool(name="aon", bufs=4))
        y_p = ctx.enter_context(tc.tile_pool(name="y", bufs=4))
        ypar_p = ctx.enter_context(tc.tile_pool(name="ypar", bufs=8))

        dq = []  # deferred closures, popped one per attention t

        # ---- projection / outproj units (heater pool; 1 PSUM bank each) ----
        def v_unit(c, i):
            ps = hp_pp.tile([128, 512], f32, tag="hp", name="vps")
            for et in range(ET):
                nc.tensor.matmul(
                    ps,
                    lhsT=xcs[c][:, et, i * 128:(i + 1) * 128],
                    rhs=wvT_sb[:, et, :],
                    start=(et == 0), stop=(et == ET - 1),
                )
            nc.vector.tensor_copy(
                out=v_aug[:, c * 4 + i, :, 0:64],
                in_=ps.rearrange("p (h d) -> p h d", h=NH),
            )

        def qk_unit(jt, c):
            # jt 0..3 = q j-tiles, 4..7 = k j-tiles
            ps = hp_pp.tile([128, 512], f32, tag="hp", name="qkps")
            qk = 0 if jt < 4 else 1
            dst = qT_sb if jt < 4 else kT_sb
            for et in range(ET):
                nc.tensor.matmul(
                    ps,
                    lhsT=wqkT_sb[:, qk, et, (jt % 4) * 128:(jt % 4 + 1) * 128],
                    rhs=xcs[c][:, et, :],
                    start=(et == 0), stop=(et == ET - 1),
                )
            nc.vector.tensor_copy(out=dst[:, jt % 4, c * 512:(c + 1) * 512], in_=ps)

        ypar = {}

        def op_part(lt, ec):
            # jt 0..2 of the tail output tiles, run as heaters once pairs
            # 0..2's window-3 aoT is final; jt 3 lands in op_fin at drain
            ps = hp_pp.tile([128, 512], f32, tag="hp", name="opps")
            for jt in range(3):
                nc.tensor.matmul(
                    ps,
                    lhsT=aoT_sb[:, jt, lt * 128:(lt + 1) * 128],
                    rhs=woT_sb[:, jt, ec * 512:(ec + 1) * 512],
                    start=(jt == 0), stop=(jt == 2),
                )
            yp = ypar_p.tile([128, 512], f32, tag="ypar")
            nc.vector.tensor_copy(out=yp, in_=ps)
            ypar[(lt, ec)] = yp

        def op_fin(lt, ec, ring):
            ps = hp_pp.tile([128, 512], f32, tag="hp", name="opps")
            nc.tensor.matmul(
                ps,
                lhsT=aoT_sb[:, 3, lt * 128:(lt + 1) * 128],
                rhs=woT_sb[:, 3, ec * 512:(ec + 1) * 512],
                start=True, stop=True,
            )
            yt = y_p.tile([128, 512], bf16, tag="y")
            nc.vector.tensor_add(out=yt, in0=ps, in1=ypar[(lt, ec)])
            ring.dma_start(
                out=y_d.ap()[lt * 128:(lt + 1) * 128, ec * 512:(ec + 1) * 512],
                in_=yt,
            )

        def op_unit(lt, ec, ring=None):
            ps = hp_pp.tile([128, 512], f32, tag="hp", name="opps")
            for jt in range(4):
                nc.tensor.matmul(
                    ps,
                    lhsT=aoT_sb[:, jt, lt * 128:(lt + 1) * 128],
                    rhs=woT_sb[:, jt, ec * 512:(ec + 1) * 512],
                    start=(jt == 0), stop=(jt == 3),
                )
            yt = y_p.tile([128, 512], bf16, tag="y")
            nc.vector.tensor_copy(out=yt, in_=ps)
            (ring or nc.gpsimd).dma_start(
                out=y_d.ap()[lt * 128:(lt + 1) * 128, ec * 512:(ec + 1) * 512],
                in_=yt,
            )

        def v_pair(c, i0):
            # two v units et-interleaved so both start on the first half of
            # a split x-chunk DMA (startup latency)
            pss = [hp_pp.tile([128, 512], f32, tag="hp", name="vps")
                   for _ in range(2)]
            for et in range(ET):
                for u, ps in enumerate(pss):
                    nc.tensor.matmul(
                        ps,
                        lhsT=xcs[c][:, et, (i0 + u) * 128:(i0 + u + 1) * 128],
                        rhs=wvT_sb[:, et, :],
                        start=(et == 0), stop=(et == ET - 1),
                    )
            for u, ps in enumerate(pss):
                nc.vector.tensor_copy(
                    out=v_aug[:, c * 4 + i0 + u, :, 0:64],
                    in_=ps.rearrange("p (h d) -> p h d", h=NH),
                )

        # ---- attention ------------------------------------------------
        def pair_unit(m, phase, hw0=(), hw1=(), last=False):
            """Heads (2m, 2m+1); phase 0 = lq windows 0,1; phase 1 = windows 2,3.

            Head A (partitions 0:64) scores land in sc[:, 0:512], head B
            (64:128) in sc[:, 512:1024]; one ACTIVATE exps both.  heaters
            (hw0/hw1 per window) are drained one per t into their own PSUM
            pool, filling PE slack under the exp stream.
            """
            for w, heaters in ((2 * phase, hw0), (2 * phase + 1, hw1)):
                lq0 = w * 512
                nt = 4 * w + 4
                # chunk-major transposed PV: the window's exp'd tiles are
                # saved and the PV runs as deferred per-chunk bursts (popped
                # 2/t in later windows).  Each lq-chunk j is one PSUM
                # accumulation group [128, 65] at column 128j; groups in a
                # bank are strictly sequential (PSUM zero regions are
                # bank-granular: an interleaved group's start would wipe its
                # siblings' partial sums).  pe chunk is the stationary
                # operand (M=128 full PE width); streaming is 65 cols vs the
                # v4 orientation's 512 — PV cycles halve.  Denominators land
                # per-partition at pv[:, 128j+64]: reciprocal_approx_fast +
                # tensor_scalar muls, then 8 PE transposes + one cast copy
                # rebuild aoT.
                pes = []
                hq = list(heaters)

                for t in range(nt):
                    off = max(0, t * 128 - lq0)
                    moff = 0 if DEBUG_FULL_SCORES else off
                    sc = sc_pp.tile([128, 1024], f32, tag="sc", name="sc")
                    for po, base in ((0, 0), (64, 512)):
                        nc.tensor.matmul(
                            sc[:, base + moff:base + 512],
                            lhsT=kT_sb[po:po + 64, m, t * 128:(t + 1) * 128],
                            rhs=qT_sb[po:po + 64, m, lq0 + moff:lq0 + 512],
                            start=True, stop=True,
                        )
                    pe = pt_p.tile([128, 1024], bf16, tag="pe", name="pe")
                    nc.scalar.activation(out=pe[:, moff:1024], in_=sc[:, moff:1024],
                                         func=Exp, scale=0.125)
                    if t >= 4 * w:  # diagonal block: zero lk > lq
                        for base in (0, 512):
                            nc.vector.tensor_mul(
                                out=pe[:, base + off:base + off + 128],
                                in0=pe[:, base + off:base + off + 128],
                                in1=diag_sb[:, 0:128],
                            )
                    if dq:
                        dq.pop(0)()
                    if dq:
                        dq.pop(0)()
                    if hq:
                        hq.pop(0)()
                    pes.append(pe)

                pvs, rcps, aons, tps = [], [], [], []

                def mk_pvc(j, pes=pes, pvs=pvs, w=w, m=m):
                    def pvc():
                        if j == 0:
                            pvs.append(pv_pp.tile([128, 512], f32, tag="pv",
                                                  name="pvA"))
                            pvs.append(pv_pp.tile([128, 512], f32, tag="pv",
                                                  name="pvB"))
                        for i, base in ((0, 0), (1, 512)):
                            pv, h = pvs[i], 2 * m + i
                            for tt in range(4 * w + j + 1):
                                nc.tensor.matmul(
                                    pv[:, 128 * j:128 * j + 65],
                                    lhsT=pes[tt][:, base + 128 * j:
                                                 base + 128 * (j + 1)],
                                    rhs=v_aug[:, tt, h, :],
                                    start=(tt == 0), stop=(tt == 4 * w + j),
                                    skip_group_check=True,
                                )
                    return pvc

                def recips(rcps=rcps, pvs=pvs):
                    for pv in pvs:
                        rcp4 = rc_p.tile([128, 4], f32, tag="rcp", name="rcp4")
                        nc.vector.reciprocal_approx_fast(
                            out=rcp4.rearrange("p (j o) -> p j o", o=1),
                            in_=pv.rearrange("p (j c) -> p j c", c=128)[:, :, 64:65],
                        )
                        rcps.append(rcp4)

                def mk_muls(i):
                    def muls(rcps=rcps, aons=aons, i=i, pvs=pvs):
                        pv = pvs[i]
                        aon = aon_p.tile([128, 256], bf16, tag="aon", name="aon")
                        for j in range(4):
                            nc.vector.tensor_scalar_mul(
                                out=aon[:, 64 * j:64 * j + 64],
                                in0=pv[:, 128 * j:128 * j + 64],
                                scalar1=rcps[i][:, j:j + 1],
                            )
                        aons.append(aon)
                    return muls

                def trans(aons=aons, tps=tps):
                    tp = hp_pp.tile([128, 512], bf16, tag="hp", name="tp")
                    for i, po in ((0, 0), (1, 64)):
                        for j in range(4):
                            nc.tensor.matmul(
                                tp[po:po + 64, 128 * j:128 * (j + 1)],
                                lhsT=aons[i][:, 64 * j:64 * j + 64],
                                rhs=ident_sb,
                                start=True, stop=True, is_transpose=True,
                                skip_group_check=True,
                            )
                    tps.append(tp)

                def fold(tps=tps, m=m, lq0=lq0):
                    nc.vector.tensor_copy(
                        out=aoT_sb[:, m, lq0:lq0 + 512], in_=tps[0])

                chain = [mk_pvc(0), mk_pvc(1), mk_pvc(2), mk_pvc(3),
                         recips, mk_muls(0), mk_muls(1), trans, fold]
                if last and w == 3:
                    for f in chain:
                        f()
                else:
                    dq.extend(chain)

        # ---- schedule -------------------------------------------------
        # warm the PE pstate on the memset tile from ~0.5us (no DMA dep);
        # the trailing memsets zero the sc slots so diagonal-strip exps
        # never see raw PSUM
        warmA = sc_pp.tile([128, 1024], f32, tag="sc", name="warmA")
        for _ in range(40):
            nc.tensor.matmul(
                warmA[:, 0:128], lhsT=warm_sb, rhs=warm_sb,
                start=True, stop=True, skip_group_check=True,
            )
        nc.vector.memset(warmA, 0.0)
        warmB = sc_pp.tile([128, 1024], f32, tag="sc", name="warmB")
        nc.vector.memset(warmB, 0.0)

        # P0: only pair-0-window-0's prerequisites run serially; everything
        # else overlaps attention as heaters.  v first (needs xc0+wvT, the
        # earliest arrivals), then q/k j-tile 0 as the weight halves land.
        v_pair(0, 0)
        v_pair(0, 2)
        qk_unit(0, 0)
        qk_unit(4, 0)

        QK = lambda jt, c: (lambda: qk_unit(jt, c))
        VU = lambda c, i: (lambda: v_unit(c, i))
        OP = lambda lt, ec: (lambda: op_unit(lt, ec))
        DX = lambda c: (lambda: nc.sync.dma_start(out=xcs[c], in_=xT_d[c].ap()))

        # A-phase: windows 0,1; heaters finish the q/k projection and kick
        # off the deferred xc2/xc3 fetches on the idle sync ring.  QK(0,1)/
        # QK(4,1) MUST run in window 0: pair-0-window-1's own scores read
        # their qT/kT columns from t=0 (xc1 lands mid-P0, well before w0-t0).
        pair_unit(0, 0, [QK(0, 1), QK(4, 1), QK(1, 0), QK(5, 0)],
                  [VU(1, 0), VU(1, 1), VU(1, 2), VU(1, 3),
                   QK(1, 1), QK(5, 1), DX(2), DX(3)])
        pair_unit(1, 0, [QK(6, 0), QK(2, 0)], [QK(6, 1), QK(2, 1)])
        pair_unit(2, 0, [QK(7, 0), QK(3, 0)], [QK(7, 1), QK(3, 1)])
        pair_unit(3, 0, [QK(0, 2), QK(0, 3), QK(4, 2)],
                  [QK(4, 3), QK(1, 2), QK(1, 3), QK(5, 2), QK(5, 3)])

        # B-phase: windows 2,3; heaters: v for lk>=1024, remaining q/k,
        # then the output projection as soon as its aoT rows are final
        woT_sb = wqk_p.tile([128, 4, E], bf16, tag="wqkT_sb", name="woT_sb")

        def load_wo():
            nc.gpsimd.dma_start(out=woT_sb, in_=woT_d.ap())

        pair_unit(0, 1, [VU(2, 0), VU(2, 1), VU(2, 2), VU(2, 3)],
                  [VU(3, 0), VU(3, 1), VU(3, 2), VU(3, 3)])
        PAR = lambda lt, ec: (lambda: op_part(lt, ec))
        # load_wo reuses the wqkT slot: it must trail the LAST wqkT readers
        # (QK(6,3)/QK(7,*)), so those are pulled forward into pair 1
        pair_unit(1, 1, [QK(2, 2), QK(2, 3), QK(6, 2)],
                  [QK(6, 3), QK(7, 2), QK(7, 3), load_wo])
        pair_unit(2, 1, [QK(3, 2), QK(3, 3)],
                  [OP(0, 0), OP(0, 1), OP(1, 0), OP(1, 1),
                   OP(6, 0), OP(6, 1), OP(7, 0), OP(7, 1)])
        pair_unit(3, 1, [OP(2, 0), OP(2, 1), OP(3, 0), OP(3, 1),
                         OP(4, 0), OP(4, 1), OP(5, 0), OP(5, 1)],
                  [PAR(12, 0), PAR(12, 1), PAR(13, 0), PAR(13, 1),
                   PAR(14, 0), PAR(14, 1), PAR(15, 0), PAR(15, 1),
                   OP(8, 0), OP(8, 1), OP(9, 0), OP(9, 1),
                   OP(10, 0), OP(10, 1), OP(11, 0), OP(11, 1)],
                  last=True)

        # drain deferred closures, then the (short) jt-3 tail of the output
        # projection: one matmul + add per tile
        while dq:
            dq.pop(0)()
        for lt in range(12, LT):
            for ec in range(2):
                op_fin(lt, ec, ring=(nc.sync if ec == 0 else nc.scalar))

        if DEBUG_DUMP:
            dbg_ao = nc.declare_dram_parameter("dbg_ao", [128, 4, L], bf16, isOutput=True)
            dbg_q = nc.declare_dram_parameter("dbg_q", [128, 4, L], bf16, isOutput=True)
            dbg_k = nc.declare_dram_parameter("dbg_k", [128, 4, L], bf16, isOutput=True)
            dbg_v = nc.declare_dram_parameter("dbg_v", [128, LT, NH, 65], bf16, isOutput=True)
            nc.sync.dma_start(out=dbg_ao.ap(), in_=aoT_sb)
            nc.sync.dma_start(out=dbg_q.ap(), in_=qT_sb)
            nc.sync.dma_start(out=dbg_k.ap(), in_=kT_sb)
            nc.sync.dma_start(out=dbg_v.ap(), in_=v_aug)

    nc.compile()
    return nc


def _shuf(a):
    """[n*128, C] -> [128, n, C] (partition-major, contiguous per partition)."""
    R, C = a.shape
    return np.ascontiguousarray(a.reshape(R // 128, 128, C).transpose(1, 0, 2))


def make_in_maps(x, w_qkv, wo):
    """Host-side sharding: 8 cores = (batch b=c//2, head-group g=c%2)."""
    import ml_dtypes
    bf = ml_dtypes.bfloat16
    x = np.asarray(x, dtype=np.float32)
    w_qkv = np.asarray(w_qkv, dtype=np.float32)
    wo = np.asarray(wo, dtype=np.float32)
    diag = np.concatenate(
        [np.triu(np.ones((128, 128), np.float32)), np.eye(128, dtype=np.float32)],
        axis=1,
    ).astype(bf)
    in_maps = []
    for c in range(8):
        b, g = c // 2, c % 2
        js = slice(g * JQ, (g + 1) * JQ)
        wq = w_qkv[0:E][js]
        wk = w_qkv[E:2 * E][js]
        wv = w_qkv[2 * E:3 * E][js]
        xT = x[b].T.astype(bf)                                   # [E, L]
        m = {
            "wqkT": np.stack([_shuf(wq.T.astype(bf)), _shuf(wk.T.astype(bf))], 1),
            "wvT": _shuf(wv.T.astype(bf)),
            "woT": _shuf(wo[:, js].T.astype(bf)),
            "diag": diag,
        }
        for cc in range(4):
            m[f"xT{cc}"] = _shuf(xT[:, cc * 512:(cc + 1) * 512])
        in_maps.append(m)
    return in_maps


def _get_nc():
    if "nc" not in _CACHE:
        _CACHE["nc"] = build_nc()
    return _CACHE["nc"]


def kernel(x, mask, w_qkv, wo, _trace=False, _trace_kwargs=None):
    from concourse.bass_utils import run_bass_kernel_spmd

    nc = _get_nc()
    in_maps = make_in_maps(x, w_qkv, wo)
    res = run_bass_kernel_spmd(
        nc, in_maps, core_ids=list(range(8)),
        trace=_trace, **(_trace_kwargs or {}),
    )
    _CACHE["last_results"] = res
    y = np.stack([
        res.results[2 * b]["y"].astype(np.float32)
        + res.results[2 * b + 1]["y"].astype(np.float32)
        for b in range(4)
    ])
    return y.astype(np.float32)
